# revision 1
# baseline (speedup 1.0000x reference)
"""Trainium2 Bass kernel for the gated-attention MIL pooling layer.

Computes, for x:[256,128,1024], v,u:[1024,512], w:[512,1]:
    h = tanh(x @ v); g = sigmoid(x @ u)
    scores = (h*g) @ w                      # [256,128,1]
    alpha  = softmax(scores, axis=0)        # over the 256 instances

Sharding: data-parallel over the batch axis (128 -> 16 per core, 8 cores).
Each core handles a [4096,1024]x[1024,512] matmul pair + a local softmax
(softmax is over instances, which live entirely on one core).

x is transposed host-side to [in_dim, m] so the Bass kernel can feed the
PE array without on-chip transposes (the contraction dim must sit on SBUF
partitions).  Matmuls run as float32r (full PE rate at moving dim >= 256,
~fp32 precision).

Written in raw Bass (explicit per-engine programs + semaphores): the
walrus build in this container rejects instructions carrying more than
one attached semaphore wait, which rules out Tile-generated sync.  All
waits here are standalone wait_ge instructions.

Startup is DMA-bandwidth-bound (v, u and the first x chunks ~ 8 MB), so
chunks 0 and 1 are streamed per-k-subtile and processed "ko-outer": each
arriving 256 KB piece immediately feeds matmuls for all 8 PSUM
accumulation groups (4 h + 4 g tiles), keeping the PE busy from the
first pieces instead of waiting ~20 us for all weights.
"""

import numpy as np

N_INST, BATCH, IN_DIM, L_DIM = 256, 128, 1024, 512
N_CORES = 8
B_LOC = BATCH // N_CORES            # 16 batch elements per core
M = N_INST * B_LOC                  # 4096 rows per core
P = 128                             # SBUF partitions
KO = IN_DIM // P                    # 8 contraction subtiles
MT = M // P                         # 32 m-tiles per core
MS = 4                              # m-tiles per x DMA chunk
NS = MT // MS                       # 8 DMA chunks

_CACHE = {}


def _build_bass():
    from contextlib import ExitStack

    import concourse.bass as bass
    import concourse.mybir as mybir

    f32 = mybir.dt.float32
    f32r = mybir.dt.float32r
    AF = mybir.ActivationFunctionType
    ALU = mybir.AluOpType

    nc = bass.Bass(
        trn_type="TRN2",
        target_bir_lowering=False,
        debug=False,
        enable_asserts=False,
    )

    xt = nc.dram_tensor("xt", [IN_DIM, M], f32r, kind="ExternalInput").ap()
    v = nc.dram_tensor("v", [IN_DIM, L_DIM], f32r, kind="ExternalInput").ap()
    u = nc.dram_tensor("u", [IN_DIM, L_DIM], f32r, kind="ExternalInput").ap()
    w_rep = nc.dram_tensor("w_rep", [P, L_DIM], f32, kind="ExternalInput").ap()
    # selb[r, c] = (r%16 == c%16): one matmul turns the per-row exp sums
    # into per-batch softmax denominators broadcast back to all 128 rows.
    selb = nc.dram_tensor("selb", [P, P], f32, kind="ExternalInput").ap()
    ident = nc.dram_tensor("ident", [P, P], f32, kind="ExternalInput").ap()
    out = nc.dram_tensor("out", [MT, P], f32, kind="ExternalOutput").ap()

    xt3 = xt.rearrange("(ko p) m -> p ko m", p=P)
    v3 = v.rearrange("(ko p) n -> p ko n", p=P)
    u3 = u.rearrange("(ko p) n -> p ko n", p=P)

    # s_pe tick after the h/g accumulation group of tile t finishes.
    # Chunks 0 and 1 run ko-outer (all four h groups complete, then all
    # four g); steady chunks alternate h/g per tile.
    def pe_h(t):
        return 8 * (t // MS) + t % MS + 1 if t < 2 * MS else 2 * t + 1

    def pe_g(t):
        return 8 * (t // MS) + t % MS + 5 if t < 2 * MS else 2 * t + 2

    # s_act tick after tanh/sigmoid of tile t (ACT always alternates
    # tanh/sigmoid per tile, even while the PE runs chunks 0/1 ko-outer).
    def act_tanh(t):
        return 2 * t + 1

    def act_sig(t):
        return 2 * t + 2

    ctx = ExitStack()
    with ctx:
        v_sb = ctx.enter_context(nc.sbuf_tensor("v_sb", [P, KO, L_DIM], f32r))
        u_sb = ctx.enter_context(nc.sbuf_tensor("u_sb", [P, KO, L_DIM], f32r))
        w_sb = ctx.enter_context(nc.sbuf_tensor("w_sb", [P, L_DIM], f32))
        selb_sb = ctx.enter_context(nc.sbuf_tensor("selb_sb", [P, P], f32))
        id_sb = ctx.enter_context(nc.sbuf_tensor("id_sb", [P, P], f32))
        x_sb = ctx.enter_context(nc.sbuf_tensor("x_sb", [P, 2, KO, MS * P], f32r))
        th_sb = ctx.enter_context(nc.sbuf_tensor("th_sb", [P, MS, L_DIM], f32))
        sg_sb = ctx.enter_context(nc.sbuf_tensor("sg_sb", [P, MS, L_DIM], f32))
        tw_sb = ctx.enter_context(nc.sbuf_tensor("tw_sb", [P, L_DIM], f32))
        z_sb = ctx.enter_context(nc.sbuf_tensor("z_sb", [P, L_DIM], f32))
        S_sb = ctx.enter_context(nc.sbuf_tensor("S_sb", [P, MT], f32))
        E_sb = ctx.enter_context(nc.sbuf_tensor("E_sb", [P, MT], f32))
        rsum_sb = ctx.enter_context(nc.sbuf_tensor("rsum_sb", [P, 1], f32))
        recip_sb = ctx.enter_context(nc.sbuf_tensor("recip_sb", [P, 1], f32))
        alpha_sb = ctx.enter_context(nc.sbuf_tensor("alpha_sb", [P, MT], f32))
        at_sb = ctx.enter_context(nc.sbuf_tensor("at_sb", [MT, P], f32))
        warm_sb = ctx.enter_context(nc.sbuf_tensor("warm_sb", [P, 4], f32))

        # All 8 PSUM banks: 4 h accumulation groups + 4 g groups (slot t%4).
        h_ps = ctx.enter_context(nc.psum_tensor("h_ps", [P, MS, L_DIM], f32))
        g_ps = ctx.enter_context(nc.psum_tensor("g_ps", [P, MS, L_DIM], f32))
        # Epilogue PSUM aliases h banks (dead by then; gated on s_act >= exp).
        rep_ps = h_ps.ap()[:, 1, :1]         # [128, 1] per-batch denominators
        at_ps = h_ps.ap()[:MT, 2, :P]        # [32, 128] transposed alpha

        s_v = [ctx.enter_context(nc.semaphore(f"s_v{k}")) for k in range(KO)]
        s_u = [ctx.enter_context(nc.semaphore(f"s_u{k}")) for k in range(KO)]
        s_x0 = [ctx.enter_context(nc.semaphore(f"s_x0k{k}")) for k in range(KO)]
        s_x1 = [ctx.enter_context(nc.semaphore(f"s_x1k{k}")) for k in range(KO)]
        s_w = ctx.enter_context(nc.semaphore("s_w"))
        s_sel = ctx.enter_context(nc.semaphore("s_sel"))
        s_id = ctx.enter_context(nc.semaphore("s_id"))
        s_x = [ctx.enter_context(nc.semaphore(f"s_x{i}")) for i in range(NS)]
        s_out = ctx.enter_context(nc.semaphore("s_out"))
        s_pe = ctx.enter_context(nc.semaphore("s_pe"))
        s_act = ctx.enter_context(nc.semaphore("s_act"))
        s_dve = ctx.enter_context(nc.semaphore("s_dve"))

        block = ctx.enter_context(nc.Block())

        # Other tick conventions:
        #   s_pe epilogue: denominator matmul -> 65, transpose -> 66.
        #   s_act: exp -> 65.
        #   s_dve: tile t: tw -> 3t+1, z -> 3t+2, reduce -> 3t+3 (96 after
        #          all); epilogue: recip -> 97, alpha -> 98, at copy -> 99.

        @block.sync
        def _(sync):
            # Startup stream: (v, x0, u) per k-subtile for chunk 0, then w,
            # then chunk 1 per k-subtile; steady chunks as whole 2MB DMAs.
            for ko in range(KO):
                sync.dma_start(
                    v_sb.ap()[:, ko, :], v3[:, ko, :]
                ).then_inc(s_v[ko], 16)
                sync.dma_start(
                    x_sb.ap()[:, 0, ko, :], xt3[:, ko, : MS * P]
                ).then_inc(s_x0[ko], 16)
                sync.dma_start(
                    u_sb.ap()[:, ko, :], u3[:, ko, :]
                ).then_inc(s_u[ko], 16)
            sync.dma_start(w_sb.ap(), w_rep[:]).then_inc(s_w, 16)
            for ko in range(KO):
                sync.dma_start(
                    x_sb.ap()[:, 1, ko, :], xt3[:, ko, MS * P : 2 * MS * P]
                ).then_inc(s_x1[ko], 16)
            sync.dma_start(selb_sb.ap(), selb[:]).then_inc(s_sel, 16)
            sync.dma_start(id_sb.ap(), ident[:]).then_inc(s_id, 16)
            for s in range(2, NS):
                # x slot s%2 free once PE finished chunk s-2
                sync.wait_ge(s_pe, 8 * (s - 1))
                sync.dma_start(
                    x_sb.ap()[:, s % 2, :, :],
                    xt3[:, :, s * MS * P : (s + 1) * MS * P],
                ).then_inc(s_x[s], 16)
            sync.wait_ge(s_dve, 3 * MT + 3)
            sync.dma_start(out[:], at_sb.ap()).then_inc(s_out, 16)
            sync.wait_ge(s_out, 16)

        @block.tensor
        def _(tensor):
            # Warm-up: five fp32 broadcast matmuls (~850ns each) keep the PE
            # busy through the DMA-bound startup so the HAM clock gate is at
            # 8/8 when the real matmuls begin.
            c0 = nc.const_aps.aps[(f32, 0.0)]
            c0b = c0.to_broadcast((P, L_DIM))
            for j in range(3):
                nc.tensor.matmul(
                    g_ps.ap()[:1, j, :], c0, c0b, start=True, stop=True
                )
            # ---- chunks 0 and 1: ko-outer over all 8 psum groups ----
            for c in range(2):
                xc = x_sb.ap()[:, c, :, :]
                for ko in range(KO):
                    if c == 0:
                        tensor.wait_ge(s_v[ko], 16)
                        tensor.wait_ge(s_x0[ko], 16)
                    else:
                        tensor.wait_ge(s_x1[ko], 16)
                    for q in range(MS):
                        if c == 1 and ko == 0:
                            # h bank q free once tanh(q) done
                            tensor.wait_ge(s_act, act_tanh(q))
                        mm = nc.tensor.matmul(
                            h_ps.ap()[:, q, :],
                            xc[:, ko, q * P : (q + 1) * P],
                            v_sb.ap()[:, ko, :],
                            start=(ko == 0),
                            stop=(ko == KO - 1),
                        )
                        if ko == KO - 1:
                            mm.then_inc(s_pe, 1)  # ticks 8c + 1..4
                    if c == 0:
                        tensor.wait_ge(s_u[ko], 16)
                    for q in range(MS):
                        if c == 1 and ko == 0:
                            # g bank q free once sigmoid(q) done
                            tensor.wait_ge(s_act, act_sig(q))
                        mm = nc.tensor.matmul(
                            g_ps.ap()[:, q, :],
                            xc[:, ko, q * P : (q + 1) * P],
                            u_sb.ap()[:, ko, :],
                            start=(ko == 0),
                            stop=(ko == KO - 1),
                        )
                        if ko == KO - 1:
                            mm.then_inc(s_pe, 1)  # ticks 8c + 5..8
            # ---- steady chunks ----
            for t in range(2 * MS, MT):
                s, q = divmod(t, MS)
                xq = x_sb.ap()[:, s % 2, :, :]
                # h bank t%4 free once tanh(t-4) done
                tensor.wait_ge(s_act, act_tanh(t - MS))
                if q == 0:
                    tensor.wait_ge(s_x[s], 16)
                for ko in range(KO):
                    mm = nc.tensor.matmul(
                        h_ps.ap()[:, t % MS, :],
                        xq[:, ko, q * P : (q + 1) * P],
                        v_sb.ap()[:, ko, :],
                        start=(ko == 0),
                        stop=(ko == KO - 1),
                    )
                mm.then_inc(s_pe, 1)  # tick 2t+1
                # g bank t%4 free once sigmoid(t-4) done
                tensor.wait_ge(s_act, act_sig(t - MS))
                for ko in range(KO):
                    mm = nc.tensor.matmul(
                        g_ps.ap()[:, t % MS, :],
                        xq[:, ko, q * P : (q + 1) * P],
                        u_sb.ap()[:, ko, :],
                        start=(ko == 0),
                        stop=(ko == KO - 1),
                    )
                mm.then_inc(s_pe, 1)  # tick 2t+2
            # ---- epilogue ----
            tensor.wait_ge(s_sel, 16)
            tensor.wait_ge(s_act, 2 * MT + 1)  # exp/rowsum done; h banks dead
            nc.tensor.matmul(
                rep_ps, selb_sb.ap(), rsum_sb.ap(), start=True, stop=True
            ).then_inc(s_pe, 1)  # -> 65: per-batch sums broadcast to rows
            tensor.wait_ge(s_id, 16)
            tensor.wait_ge(s_dve, 3 * MT + 2)  # alpha ready
            nc.tensor.transpose(at_ps, alpha_sb.ap(), id_sb.ap()).then_inc(
                s_pe, 1
            )  # -> 66

        @block.scalar
        def _(scalar):
            # Dummy activations: pre-load the tanh/sigmoid/exp tables during
            # the DMA-bound startup.
            c0 = nc.const_aps.aps[(f32, 0.0)]
            for j, fn in enumerate((AF.Tanh, AF.Sigmoid, AF.Exp)):
                nc.scalar.activation(warm_sb.ap()[:, j : j + 1], c0, fn)

            def tanh_t(t):
                scalar.wait_ge(s_pe, pe_h(t))
                if t >= MS:
                    scalar.wait_ge(s_dve, 3 * (t - MS) + 1)  # th slot free
                nc.scalar.activation(
                    th_sb.ap()[:, t % MS, :], h_ps.ap()[:, t % MS, :], AF.Tanh
                ).then_inc(s_act, 1)

            def sig_t(t):
                scalar.wait_ge(s_pe, pe_g(t))
                if t >= MS:
                    scalar.wait_ge(s_dve, 3 * (t - MS) + 2)  # sg slot free
                nc.scalar.activation(
                    sg_sb.ap()[:, t % MS, :], g_ps.ap()[:, t % MS, :], AF.Sigmoid
                ).then_inc(s_act, 1)

            for t in range(MT):
                tanh_t(t)
                sig_t(t)
            # Dummy exp BEFORE the final wait: walrus re-emits the exp
            # table load at the sigmoid->exp transition, so trigger it here
            # where it overlaps the DVE tail instead of the critical path.
            nc.scalar.activation(warm_sb.ap()[:, 3:4], c0, AF.Exp)
            # softmax numerators + row sums (no max-subtraction needed:
            # |score| <= sum|w| ~ 28, exp stays well inside fp32 range)
            scalar.wait_ge(s_dve, 3 * MT)  # S complete
            nc.scalar.activation(
                E_sb.ap(), S_sb.ap(), AF.Exp, accum_out=rsum_sb.ap()
            ).then_inc(s_act, 1)  # -> 65

        @block.vector
        def _(vector):
            vector.wait_ge(s_w, 16)
            for t in range(MT):
                vector.wait_ge(s_act, act_tanh(t))
                nc.vector.tensor_tensor(
                    tw_sb.ap(), th_sb.ap()[:, t % MS, :], w_sb.ap(), ALU.mult
                ).then_inc(s_dve, 1)
                vector.wait_ge(s_act, act_sig(t))
                vector.wait_ge(s_dve, 3 * t + 1)  # tw RAW (same-engine order)
                nc.vector.tensor_tensor(
                    z_sb.ap(), tw_sb.ap(), sg_sb.ap()[:, t % MS, :], ALU.mult
                ).then_inc(s_dve, 1)
                vector.wait_ge(s_dve, 3 * t + 2)  # z RAW
                nc.vector.tensor_reduce(
                    S_sb.ap()[:, t : t + 1],
                    z_sb.ap(),
                    axis=mybir.AxisListType.X,
                    op=ALU.add,
                ).then_inc(s_dve, 1)
            # epilogue
            vector.wait_ge(s_pe, 2 * MT + 1)  # rep_ps (denominators) ready
            nc.vector.reciprocal(recip_sb.ap(), rep_ps).then_inc(s_dve, 1)  # 97
            vector.wait_ge(s_act, 2 * MT + 1)  # E ready
            vector.wait_ge(s_dve, 3 * MT + 1)  # recip_sb RAW
            nc.vector.tensor_scalar_mul(
                alpha_sb.ap(), E_sb.ap(), recip_sb.ap()
            ).then_inc(s_dve, 1)  # 98
            vector.wait_ge(s_pe, 2 * MT + 2)  # at_ps ready
            nc.vector.tensor_copy(at_sb.ap(), at_ps).then_inc(s_dve, 1)  # 99

    return nc


def _host_inputs(x, v, u, w):
    """Build the per-core input maps (host-side shard + layout prep)."""
    x = np.asarray(x, dtype=np.float32)
    v = np.ascontiguousarray(np.asarray(v, dtype=np.float32))
    u = np.ascontiguousarray(np.asarray(u, dtype=np.float32))
    w = np.asarray(w, dtype=np.float32).reshape(L_DIM)

    w_rep = np.ascontiguousarray(np.broadcast_to(w, (P, L_DIM)))
    selb = (
        np.arange(P)[:, None] % B_LOC == np.arange(P)[None, :] % B_LOC
    ).astype(np.float32)
    ident = np.eye(P, dtype=np.float32)

    common = {"v": v, "u": u, "w_rep": w_rep, "selb": selb, "ident": ident}
    in_maps = []
    for c in range(N_CORES):
        xc = x[:, c * B_LOC : (c + 1) * B_LOC, :].reshape(M, IN_DIM)
        xtc = np.ascontiguousarray(xc.T)  # [IN_DIM, M]
        in_maps.append({"xt": xtc, **common})
    return in_maps


def kernel(x, v, u, w):
    from concourse.bass_utils import run_bass_kernel_spmd

    if "nc" not in _CACHE:
        _CACHE["nc"] = _build_bass()
    nc = _CACHE["nc"]

    in_maps = _host_inputs(x, v, u, w)
    res = run_bass_kernel_spmd(nc, in_maps, core_ids=list(range(N_CORES)))
    _CACHE["last_result"] = res

    parts = []
    for c in range(N_CORES):
        a = res.results[c]["out"]  # [32, 128], flat index = m = i*16 + b_loc
        parts.append(a.reshape(N_INST, B_LOC))
    full = np.concatenate(parts, axis=1)[:, :, None]
    return np.ascontiguousarray(full.astype(np.float32))



# revision 20
# speedup vs baseline: 1.0976x; 1.0976x over previous
"""Trainium2 Bass kernel for the gated-attention MIL pooling layer.

Computes, for x:[256,128,1024], v,u:[1024,512], w:[512,1]:
    h = tanh(x @ v); g = sigmoid(x @ u)
    scores = (h*g) @ w                      # [256,128,1]
    alpha  = softmax(scores, axis=0)        # over the 256 instances

Sharding: data-parallel over the batch axis (128 -> 16 per core, 8 cores).
Each core handles a [4096,1024]x[1024,512] matmul pair + a local softmax
(softmax is over instances, which live entirely on one core).

Matmul strategy: fp8e4m3 DoubleRow matmuls (0.5 PE cycles/row = 4x the
fp32r rate).  Plain fp8 on both paths measures rel-err 2.2e-2 (over the
2e-2 gate), and the tanh path dominates the error budget, so the h path
is error-compensated: x = x_hi + x_lo and v = v_hi + v_lo (each part
e4m3), computing x_hi*v_hi + x_lo*v_hi + x_hi*v_lo (lo*lo dropped).
The g path uses plain x_hi @ u8.  Measured end-to-end rel-err ~8e-3.
Per m-tile of 128 rows this is 12 + 4 DoubleRow matmuls (each contracts
K=256 via two 128-deep slots) = 1707 ns of PE time; 32 m-tiles = 54.6 us
per core, vs 109 us at fp32r.

v,u are pre-scaled by 16 host-side so their small uniform entries stay
in e4m3 normal range; the activation applies scale=1/16 on the PSUM
input (out = func(in*scale)).

Activations write bf16 so the DVE elementwise ops run in 2x/4x mode:
tw = th*w (tensor_tensor) then one fused tensor_tensor_reduce gives
z = tw*sg and S[:,t] = rowsum(z) in a single pass.

Epilogue (unchanged from the fp32r version): exp with accumulated row
sums, a selb matmul to broadcast per-batch softmax denominators, DVE
reciprocal + scale, PE transpose, DMA out.

Raw Bass (explicit per-engine programs + standalone wait_ge semaphores;
the walrus build rejects >1 attached wait per instruction).

DMA: x (hi+lo interleaved fp8, 8 MB/core) streams on the SP queue in 8
chunks (chunk 0 split in four 2-ko pieces so the PE can start early);
weights go on the ACT queue (v_dup, v_lo) and DVE queue (u, w) in
parallel; selb/ident on the gpsimd queue.  Warm-up matmuls keep the PE
p-state ramping through the DMA-bound startup.
"""

import numpy as np

N_INST, BATCH, IN_DIM, L_DIM = 256, 128, 1024, 512
N_CORES = 8
B_LOC = BATCH // N_CORES            # 16 batch elements per core
M = N_INST * B_LOC                  # 4096 rows per core
P = 128                             # SBUF partitions
KO = IN_DIM // P                    # 8 contraction subtiles
KP = KO // 2                        # 4 DoubleRow k-pair groups
MT = M // P                         # 32 m-tiles per core
MS = 4                              # m-tiles per x DMA chunk
NS = MT // MS                       # 8 DMA chunks
SV = 16.0                           # host-side scale on v,u (e4m3 range)

_CACHE = {}


def _build_bass():
    from contextlib import ExitStack

    import concourse.bass as bass
    import concourse.mybir as mybir

    f32 = mybir.dt.float32
    bf16 = mybir.dt.bfloat16
    f8 = mybir.dt.float8e4
    AF = mybir.ActivationFunctionType
    ALU = mybir.AluOpType
    DR = mybir.MatmulPerfMode.DoubleRow

    nc = bass.Bass(
        trn_type="TRN2",
        target_bir_lowering=False,
        debug=False,
        enable_asserts=False,
    )

    # x hi/lo interleaved: [p, chunk, ko, {hi,lo}, m-in-chunk]
    xq = nc.dram_tensor("xq", [P, NS, KO, 2, MS * P], f8, kind="ExternalInput").ap()
    # v_hi duplicated on the slot axis (DoubleRow rhs needs it twice)
    vq = nc.dram_tensor("vq", [P, KO, 2, L_DIM], f8, kind="ExternalInput").ap()
    vl = nc.dram_tensor("vl", [P, KO, L_DIM], f8, kind="ExternalInput").ap()
    uq = nc.dram_tensor("uq", [P, KO, L_DIM], f8, kind="ExternalInput").ap()
    w_rep = nc.dram_tensor("w_rep", [P, L_DIM], bf16, kind="ExternalInput").ap()
    # selb[r, c] = (r%16 == c%16): one matmul turns the per-row exp sums
    # into per-batch softmax denominators broadcast back to all 128 rows.
    selb = nc.dram_tensor("selb", [P, P], f32, kind="ExternalInput").ap()
    ident = nc.dram_tensor("ident", [P, P], f32, kind="ExternalInput").ap()
    out = nc.dram_tensor("out", [MT, P], f32, kind="ExternalOutput").ap()

    ctx = ExitStack()
    with ctx:
        v_sb = ctx.enter_context(nc.sbuf_tensor("v_sb", [P, KO, 2, L_DIM], f8))
        vl_sb = ctx.enter_context(nc.sbuf_tensor("vl_sb", [P, KO, L_DIM], f8))
        u_sb = ctx.enter_context(nc.sbuf_tensor("u_sb", [P, KO, L_DIM], f8))
        w_sb = ctx.enter_context(nc.sbuf_tensor("w_sb", [P, L_DIM], bf16))
        selb_sb = ctx.enter_context(nc.sbuf_tensor("selb_sb", [P, P], f32))
        id_sb = ctx.enter_context(nc.sbuf_tensor("id_sb", [P, P], f32))
        x_sb = ctx.enter_context(
            nc.sbuf_tensor("x_sb", [P, 2, KO, 2, MS * P], f8)
        )
        th_sb = ctx.enter_context(nc.sbuf_tensor("th_sb", [P, MS, L_DIM], bf16))
        sg_sb = ctx.enter_context(nc.sbuf_tensor("sg_sb", [P, MS, L_DIM], bf16))
        tw_sb = ctx.enter_context(nc.sbuf_tensor("tw_sb", [P, L_DIM], bf16))
        z_sb = ctx.enter_context(nc.sbuf_tensor("z_sb", [P, L_DIM], bf16))
        S_sb = ctx.enter_context(nc.sbuf_tensor("S_sb", [P, MT], f32))
        E_sb = ctx.enter_context(nc.sbuf_tensor("E_sb", [P, MT], f32))
        rsum_sb = ctx.enter_context(nc.sbuf_tensor("rsum_sb", [P, 1], f32))
        recip_sb = ctx.enter_context(nc.sbuf_tensor("recip_sb", [P, 1], f32))
        alpha_sb = ctx.enter_context(nc.sbuf_tensor("alpha_sb", [P, MT], f32))
        at_sb = ctx.enter_context(nc.sbuf_tensor("at_sb", [MT, P], f32))
        warm_sb = ctx.enter_context(nc.sbuf_tensor("warm_sb", [P, 4], f32))

        # All 8 PSUM banks: 4 h accumulation groups + 4 g groups (slot t%4).
        h_ps = ctx.enter_context(nc.psum_tensor("h_ps", [P, MS, L_DIM], f32))
        g_ps = ctx.enter_context(nc.psum_tensor("g_ps", [P, MS, L_DIM], f32))
        # Epilogue PSUM aliases h banks (dead by then; gated on s_act/s_dve).
        rep_ps = h_ps.ap()[:, 1, :1]         # [128, 1] per-batch denominators
        at_ps = h_ps.ap()[:MT, 2, :P]        # [32, 128] transposed alpha

        s_v = [ctx.enter_context(nc.semaphore(f"s_v{i}")) for i in range(2)]
        s_vl = ctx.enter_context(nc.semaphore("s_vl"))
        s_u = ctx.enter_context(nc.semaphore("s_u"))
        s_w = ctx.enter_context(nc.semaphore("s_w"))
        s_sel = ctx.enter_context(nc.semaphore("s_sel"))
        s_id = ctx.enter_context(nc.semaphore("s_id"))
        s_x0 = [
            ctx.enter_context(nc.semaphore(f"s_x0p{i}")) for i in range(4)
        ]                                                 # chunk-0 pieces
        s_x = [ctx.enter_context(nc.semaphore(f"s_x{i}")) for i in range(1, NS)]
        s_out = ctx.enter_context(nc.semaphore("s_out"))
        s_pe = ctx.enter_context(nc.semaphore("s_pe"))
        s_act = ctx.enter_context(nc.semaphore("s_act"))
        s_dve = ctx.enter_context(nc.semaphore("s_dve"))

        block = ctx.enter_context(nc.Block())

        # Tick conventions:
        #   s_pe: chunk 0: h groups -> 1..4, g groups -> 5..8;
        #         steady tile t: h -> 2t+1, g -> 2t+2;
        #         epilogue: denominator matmul -> 65, transpose -> 66.
        #   s_act: tile t: tanh -> 2t+1, sigmoid -> 2t+2; exp -> 65.
        #   s_dve: tile t: tw -> 3t+1, z -> 3t+2, reduce -> 3t+3;
        #          epilogue: recip -> 97, alpha -> 98, at copy -> 99.
        def pe_h(t):
            return t + 1 if t < MS else 2 * t + 1

        def pe_g(t):
            return t + 5 if t < MS else 2 * t + 2

        def act_tanh(t):
            return 2 * t + 1

        def act_sig(t):
            return 2 * t + 2

        # ---- DMA programs (SP queue: x stream; ACT queue: weights) ----

        @block.sync
        def _(sync):
            # x chunk 0 in four 2-ko pieces so the PE can start ~2.5us in.
            for i in range(4):
                sync.dma_start(
                    x_sb.ap()[:, 0, 2 * i : 2 * i + 2],
                    xq[:, 0, 2 * i : 2 * i + 2],
                ).then_inc(s_x0[i], 16)
            for s in range(1, NS):
                if s >= 2:
                    # x slot s%2 free once PE finished chunk s-2
                    sync.wait_ge(s_pe, 8 * s - 8)
                sync.dma_start(
                    x_sb.ap()[:, s % 2], xq[:, s]
                ).then_inc(s_x[s - 1], 16)
            # epilogue constants (needed ~15us after the last x chunk issue)
            sync.dma_start(selb_sb.ap(), selb[:]).then_inc(s_sel, 16)
            sync.dma_start(id_sb.ap(), ident[:]).then_inc(s_id, 16)
            sync.wait_ge(s_dve, 3 * MT + 3)
            sync.dma_start(out[:], at_sb.ap()).then_inc(s_out, 16)
            sync.wait_ge(s_out, 16)

        # ---- PE ----

        @block.tensor
        def _(tensor):
            # Warm-up: fp32 broadcast matmuls keep the PE busy through the
            # DMA-bound startup so the p-state ramp overlaps it.
            c0 = nc.const_aps.aps[(f32, 0.0)]
            c0b = c0.to_broadcast((P, L_DIM))
            for j in range(3):
                nc.tensor.matmul(
                    g_ps.ap()[:1, j, :], c0, c0b, start=True, stop=True
                )

            def h_group(xv, t, chunk0_k=None):
                """Emit the 12 DR matmuls of tile t's h group (or, for
                chunk 0, just the DR1s of one k / the DR3+stop tail)."""
                q = t % MS
                cols = slice(q * P, (q + 1) * P)
                if chunk0_k is None:
                    ks = range(KO)
                else:
                    ks = chunk0_k
                for k in ks:
                    nc.tensor.matmul(
                        h_ps.ap()[:, q, :],
                        xv[:, k, :, cols],
                        v_sb.ap()[:, k, :, :],
                        start=(k == 0),
                        stop=False,
                        perf_mode=DR,
                    )

            def h_tail(xv, t):
                q = t % MS
                cols = slice(q * P, (q + 1) * P)
                for kp in range(KP):
                    mm = nc.tensor.matmul(
                        h_ps.ap()[:, q, :],
                        xv[:, 2 * kp : 2 * kp + 2, 0, cols],
                        vl_sb.ap()[:, 2 * kp : 2 * kp + 2, :],
                        start=False,
                        stop=(kp == KP - 1),
                        perf_mode=DR,
                    )
                mm.then_inc(s_pe, 1)

            def g_group(xv, t):
                q = t % MS
                cols = slice(q * P, (q + 1) * P)
                for kp in range(KP):
                    mm = nc.tensor.matmul(
                        g_ps.ap()[:, q, :],
                        xv[:, 2 * kp : 2 * kp + 2, 0, cols],
                        u_sb.ap()[:, 2 * kp : 2 * kp + 2, :],
                        start=(kp == 0),
                        stop=(kp == KP - 1),
                        perf_mode=DR,
                    )
                mm.then_inc(s_pe, 1)

            # ---- chunk 0: ko-outer so compute starts as pieces arrive ----
            xv = x_sb.ap()[:, 0]
            for k in range(KO):
                if k % 2 == 0:
                    tensor.wait_ge(s_v[0 if k < 2 else 1], 16)
                    tensor.wait_ge(s_x0[k // 2], 16)
                for q in range(MS):
                    h_group(xv, q, chunk0_k=[k])
            tensor.wait_ge(s_vl, 16)
            for q in range(MS):
                h_tail(xv, q)            # ticks 1..4
            tensor.wait_ge(s_u, 16)
            for q in range(MS):
                g_group(xv, q)           # ticks 5..8
            # ---- steady chunks ----
            for t in range(MS, MT):
                s, q = divmod(t, MS)
                xv = x_sb.ap()[:, s % 2]
                # h bank q free once tanh(t-4) done
                tensor.wait_ge(s_act, act_tanh(t - MS))
                if q == 0:
                    tensor.wait_ge(s_x[s - 1], 16)
                h_group(xv, t)
                h_tail(xv, t)            # tick 2t+1
                # g bank q free once sigmoid(t-4) done
                tensor.wait_ge(s_act, act_sig(t - MS))
                g_group(xv, t)           # tick 2t+2
            # ---- epilogue ----
            tensor.wait_ge(s_sel, 16)
            tensor.wait_ge(s_act, 2 * MT + 1)  # exp/rowsum done; h banks dead
            nc.tensor.matmul(
                rep_ps, selb_sb.ap(), rsum_sb.ap(), start=True, stop=True
            ).then_inc(s_pe, 1)  # -> 65: per-batch sums broadcast to rows
            tensor.wait_ge(s_id, 16)
            tensor.wait_ge(s_dve, 3 * MT + 2)  # alpha ready
            nc.tensor.transpose(at_ps, alpha_sb.ap(), id_sb.ap()).then_inc(
                s_pe, 1
            )  # -> 66

        # ---- ACT ----

        @block.scalar
        def _(scalar):
            # Weight DMAs on the ACT HWDGE queue, in order of first use:
            # v_dup ko0-1 (PE chunk-0 start), v_dup ko2-7, v_lo (h tails),
            # w (first DVE tw), u (g groups).
            scalar.dma_start(v_sb.ap()[:, :2], vq[:, :2]).then_inc(s_v[0], 16)
            scalar.dma_start(v_sb.ap()[:, 2:], vq[:, 2:]).then_inc(s_v[1], 16)
            scalar.dma_start(vl_sb.ap(), vl[:]).then_inc(s_vl, 16)
            scalar.dma_start(w_sb.ap(), w_rep[:]).then_inc(s_w, 16)
            scalar.dma_start(u_sb.ap(), uq[:]).then_inc(s_u, 16)
            # Dummy activations: pre-load the tanh/sigmoid tables during
            # the DMA-bound startup.
            c0 = nc.const_aps.aps[(f32, 0.0)]
            for j, fn in enumerate((AF.Tanh, AF.Sigmoid, AF.Exp)):
                nc.scalar.activation(warm_sb.ap()[:, j : j + 1], c0, fn)

            inv = 1.0 / SV
            for t in range(MT):
                q = t % MS
                scalar.wait_ge(s_pe, pe_h(t))
                if t >= MS:
                    scalar.wait_ge(s_dve, 3 * (t - MS) + 1)  # th slot free
                nc.scalar.activation(
                    th_sb.ap()[:, q, :], h_ps.ap()[:, q, :], AF.Tanh, scale=inv
                ).then_inc(s_act, 1)
                scalar.wait_ge(s_pe, pe_g(t))
                if t >= MS:
                    scalar.wait_ge(s_dve, 3 * (t - MS) + 2)  # sg slot free
                nc.scalar.activation(
                    sg_sb.ap()[:, q, :], g_ps.ap()[:, q, :], AF.Sigmoid, scale=inv
                ).then_inc(s_act, 1)
            # Dummy exp BEFORE the final wait: walrus re-emits the exp
            # table load at the sigmoid->exp transition, so trigger it here
            # where it overlaps the DVE tail instead of the critical path.
            nc.scalar.activation(warm_sb.ap()[:, 3:4], c0, AF.Exp)
            # softmax numerators + row sums (no max-subtraction needed:
            # |score| <= sum|w| ~ 28, exp stays well inside fp32 range)
            scalar.wait_ge(s_dve, 3 * MT)  # S complete
            nc.scalar.activation(
                E_sb.ap(), S_sb.ap(), AF.Exp, accum_out=rsum_sb.ap()
            ).then_inc(s_act, 1)  # -> 65

        # ---- DVE ----

        @block.vector
        def _(vector):
            vector.wait_ge(s_w, 16)
            for t in range(MT):
                q = t % MS
                vector.wait_ge(s_act, act_tanh(t))
                if t > 0:
                    vector.wait_ge(s_dve, 3 * t - 1)  # tw WAR vs z(t-1) read
                nc.vector.tensor_tensor(
                    tw_sb.ap(), th_sb.ap()[:, q, :], w_sb.ap(), ALU.mult
                ).then_inc(s_dve, 1)
                vector.wait_ge(s_act, act_sig(t))
                vector.wait_ge(s_dve, 3 * t + 1)  # tw RAW (same-engine order)
                nc.vector.tensor_tensor(
                    z_sb.ap(), tw_sb.ap(), sg_sb.ap()[:, q, :], ALU.mult
                ).then_inc(s_dve, 1)
                vector.wait_ge(s_dve, 3 * t + 2)  # z RAW
                nc.vector.tensor_reduce(
                    S_sb.ap()[:, t : t + 1],
                    z_sb.ap(),
                    axis=mybir.AxisListType.X,
                    op=ALU.add,
                ).then_inc(s_dve, 1)
            # epilogue
            vector.wait_ge(s_pe, 2 * MT + 1)  # rep_ps (denominators) ready
            nc.vector.reciprocal(recip_sb.ap(), rep_ps).then_inc(s_dve, 1)
            vector.wait_ge(s_act, 2 * MT + 1)  # E ready
            vector.wait_ge(s_dve, 3 * MT + 1)  # recip_sb RAW (same engine)
            nc.vector.tensor_scalar_mul(
                alpha_sb.ap(), E_sb.ap(), recip_sb.ap()
            ).then_inc(s_dve, 1)  # -> 98
            vector.wait_ge(s_pe, 2 * MT + 2)  # at_ps ready
            nc.vector.tensor_copy(at_sb.ap(), at_ps).then_inc(s_dve, 1)  # 99

    return nc


def _host_inputs(x, v, u, w):
    """Build the per-core input maps (host-side shard + fp8 layout prep)."""
    import ml_dtypes

    f8 = ml_dtypes.float8_e4m3
    bf = ml_dtypes.bfloat16

    x = np.asarray(x, dtype=np.float32)
    v = np.asarray(v, dtype=np.float32)
    u = np.asarray(u, dtype=np.float32)
    w = np.asarray(w, dtype=np.float32).reshape(L_DIM)

    vs = v * SV
    v_hi = vs.astype(f8)
    v_lo = (vs - v_hi.astype(np.float32)).astype(f8)
    u8 = (u * SV).astype(f8)

    # [IN_DIM, L] -> [p, ko, L]
    def to_pko(a):
        return np.ascontiguousarray(
            a.reshape(KO, P, L_DIM).transpose(1, 0, 2)
        )

    vq = to_pko(v_hi)                       # [P, KO, L]
    vq = np.ascontiguousarray(
        np.broadcast_to(vq[:, :, None, :], (P, KO, 2, L_DIM))
    )                                       # duplicate on slot axis
    vl = to_pko(v_lo)
    uq = to_pko(u8)

    w_rep = np.ascontiguousarray(np.broadcast_to(w.astype(bf), (P, L_DIM)))
    selb = (
        np.arange(P)[:, None] % B_LOC == np.arange(P)[None, :] % B_LOC
    ).astype(np.float32)
    ident = np.eye(P, dtype=np.float32)

    common = {
        "vq": vq, "vl": vl, "uq": uq, "w_rep": w_rep,
        "selb": selb, "ident": ident,
    }
    in_maps = []
    for c in range(N_CORES):
        xc = x[:, c * B_LOC : (c + 1) * B_LOC, :].reshape(M, IN_DIM)
        xt = np.ascontiguousarray(xc.T)     # [IN_DIM, M] f32
        x_hi = xt.astype(f8)
        x_lo = (xt - x_hi.astype(np.float32)).astype(f8)
        # [j, ko, p, s, m] -> [p, s, ko, j, m]
        xs = np.stack(
            [
                x_hi.reshape(KO, P, NS, MS * P),
                x_lo.reshape(KO, P, NS, MS * P),
            ]
        )
        xqc = np.ascontiguousarray(xs.transpose(2, 3, 1, 0, 4))
        in_maps.append({"xq": xqc, **common})
    return in_maps


def kernel(x, v, u, w):
    from concourse.bass_utils import run_bass_kernel_spmd

    if "nc" not in _CACHE:
        _CACHE["nc"] = _build_bass()
    nc = _CACHE["nc"]

    in_maps = _host_inputs(x, v, u, w)
    res = run_bass_kernel_spmd(nc, in_maps, core_ids=list(range(N_CORES)))
    _CACHE["last_result"] = res

    parts = []
    for c in range(N_CORES):
        a = res.results[c]["out"]  # [32, 128], flat index = m = i*16 + b_loc
        parts.append(a.reshape(N_INST, B_LOC))
    full = np.concatenate(parts, axis=1)[:, :, None]
    return np.ascontiguousarray(full.astype(np.float32))


# revision 21
# speedup vs baseline: 1.3797x; 1.2570x over previous
"""Trainium2 Bass kernel for the gated-attention MIL pooling layer.

Computes, for x:[256,128,1024], v,u:[1024,512], w:[512,1]:
    h = tanh(x @ v); g = sigmoid(x @ u)
    scores = (h*g) @ w                      # [256,128,1]
    alpha  = softmax(scores, axis=0)        # over the 256 instances

Sharding: data-parallel over the batch axis (128 -> 16 per core, 8 cores).
Each core handles a [4096,1024]x[1024,512] matmul pair + a local softmax
(softmax is over instances, which live entirely on one core).

Matmul strategy (HW-measured rates: one 512-row matmul instruction costs
~216 ns regardless of dtype; fp8e4m3 DoubleRow contracts K=256 per
instruction vs K=128 for bf16 -> fp8 is 2x bf16):
  - tanh path in bf16 (8 matmuls per 128-row m-tile).  The tanh path
    dominates the softmax error budget (fp8 here measures 2.1e-2 alone,
    over the 2e-2 gate), so it gets the accurate dtype.
  - sigmoid path in plain fp8e4m3 DoubleRow (4 matmuls per m-tile,
    K=256 each via two 128-deep slot pairs).  g-path-only fp8 measures
    7.9e-3 end-to-end - 2.5x under the gate.
  12 instructions per m-tile = 2.6 us -> ~83 us PE per core, vs 110 us
  all-bf16/fp32r.  u is pre-scaled by 16 host-side so its small uniform
  entries stay in e4m3 normal range; the sigmoid applies scale=1/16 on
  the PSUM input (out = func(in*scale)).

Activations write bf16 so the DVE elementwise ops run in 2x mode:
tw = th*w, z = tw*sg, S[:,t] = rowsum(z) (three standard DVE ops; the
fused tensor_tensor_reduce/custom-DVE ops fail this walrus build's
codegen with "ISA wrong length").

Epilogue: exp with accumulated row sums, a selb matmul to broadcast
per-batch softmax denominators, DVE reciprocal + scale, PE transpose,
DMA out.

Raw Bass (explicit per-engine programs + standalone wait_ge semaphores;
the walrus build rejects >1 attached wait per instruction).

DMA: x streams as bf16 (tanh path, 8 MB) + fp8 (sigmoid path, 4 MB) on
the SP queue in 8 chunks (chunk 0 split in four 2-ko bf16 pieces so the
PE can start early); v/w/u go on the ACT HWDGE queue in parallel.
Warm-up matmuls keep the PE p-state ramping through the DMA-bound
startup.
"""

import numpy as np

N_INST, BATCH, IN_DIM, L_DIM = 256, 128, 1024, 512
N_CORES = 8
B_LOC = BATCH // N_CORES            # 16 batch elements per core
M = N_INST * B_LOC                  # 4096 rows per core
P = 128                             # SBUF partitions
KO = IN_DIM // P                    # 8 contraction subtiles
KP = KO // 2                        # 4 DoubleRow k-pair groups
MT = M // P                         # 32 m-tiles per core
MS = 4                              # m-tiles per x DMA chunk
NS = MT // MS                       # 8 DMA chunks
SU = 16.0                           # host-side scale on u (e4m3 range)

_CACHE = {}


def _build_bass():
    from contextlib import ExitStack

    import concourse.bass as bass
    import concourse.mybir as mybir

    f32 = mybir.dt.float32
    bf16 = mybir.dt.bfloat16
    f8 = mybir.dt.float8e4
    AF = mybir.ActivationFunctionType
    ALU = mybir.AluOpType
    DR = mybir.MatmulPerfMode.DoubleRow

    nc = bass.Bass(
        trn_type="TRN2",
        target_bir_lowering=False,
        debug=False,
        enable_asserts=False,
    )

    # x in two precisions: bf16 for the tanh path, fp8 for the sigmoid path
    xb = nc.dram_tensor("xb", [P, NS, KO, MS * P], bf16, kind="ExternalInput").ap()
    x8 = nc.dram_tensor("x8", [P, NS, KO, MS * P], f8, kind="ExternalInput").ap()
    vb = nc.dram_tensor("vb", [P, KO, L_DIM], bf16, kind="ExternalInput").ap()
    uq = nc.dram_tensor("uq", [P, KO, L_DIM], f8, kind="ExternalInput").ap()
    w_rep = nc.dram_tensor("w_rep", [P, L_DIM], bf16, kind="ExternalInput").ap()
    # selb[r, c] = (r%16 == c%16): one matmul turns the per-row exp sums
    # into per-batch softmax denominators broadcast back to all 128 rows.
    selb = nc.dram_tensor("selb", [P, P], f32, kind="ExternalInput").ap()
    ident = nc.dram_tensor("ident", [P, P], f32, kind="ExternalInput").ap()
    out = nc.dram_tensor("out", [MT, P], f32, kind="ExternalOutput").ap()

    ctx = ExitStack()
    with ctx:
        v_sb = ctx.enter_context(nc.sbuf_tensor("v_sb", [P, KO, L_DIM], bf16))
        u_sb = ctx.enter_context(nc.sbuf_tensor("u_sb", [P, KO, L_DIM], f8))
        w_sb = ctx.enter_context(nc.sbuf_tensor("w_sb", [P, L_DIM], bf16))
        selb_sb = ctx.enter_context(nc.sbuf_tensor("selb_sb", [P, P], f32))
        id_sb = ctx.enter_context(nc.sbuf_tensor("id_sb", [P, P], f32))
        xb_sb = ctx.enter_context(
            nc.sbuf_tensor("xb_sb", [P, 2, KO, MS * P], bf16)
        )
        x8_sb = ctx.enter_context(
            nc.sbuf_tensor("x8_sb", [P, 2, KO, MS * P], f8)
        )
        th_sb = ctx.enter_context(nc.sbuf_tensor("th_sb", [P, MS, L_DIM], bf16))
        sg_sb = ctx.enter_context(nc.sbuf_tensor("sg_sb", [P, MS, L_DIM], bf16))
        tw_sb = ctx.enter_context(nc.sbuf_tensor("tw_sb", [P, L_DIM], bf16))
        z_sb = ctx.enter_context(nc.sbuf_tensor("z_sb", [P, L_DIM], bf16))
        S_sb = ctx.enter_context(nc.sbuf_tensor("S_sb", [P, MT], f32))
        E_sb = ctx.enter_context(nc.sbuf_tensor("E_sb", [P, MT], f32))
        rsum_sb = ctx.enter_context(nc.sbuf_tensor("rsum_sb", [P, 1], f32))
        recip_sb = ctx.enter_context(nc.sbuf_tensor("recip_sb", [P, 1], f32))
        alpha_sb = ctx.enter_context(nc.sbuf_tensor("alpha_sb", [P, MT], f32))
        at_sb = ctx.enter_context(nc.sbuf_tensor("at_sb", [MT, P], f32))
        warm_sb = ctx.enter_context(nc.sbuf_tensor("warm_sb", [P, 4], f32))

        # All 8 PSUM banks: 4 h accumulation groups + 4 g groups (slot t%4).
        h_ps = ctx.enter_context(nc.psum_tensor("h_ps", [P, MS, L_DIM], f32))
        g_ps = ctx.enter_context(nc.psum_tensor("g_ps", [P, MS, L_DIM], f32))
        # Epilogue PSUM aliases h banks (dead by then; gated on s_act/s_dve).
        rep_ps = h_ps.ap()[:, 1, :1]         # [128, 1] per-batch denominators
        at_ps = h_ps.ap()[:MT, 2, :P]        # [32, 128] transposed alpha

        s_v = [ctx.enter_context(nc.semaphore(f"s_v{i}")) for i in range(2)]
        s_u = ctx.enter_context(nc.semaphore("s_u"))
        s_w = ctx.enter_context(nc.semaphore("s_w"))
        s_sel = ctx.enter_context(nc.semaphore("s_sel"))
        s_id = ctx.enter_context(nc.semaphore("s_id"))
        s_x0 = [
            ctx.enter_context(nc.semaphore(f"s_x0p{i}")) for i in range(4)
        ]                                                 # chunk-0 bf16 pieces
        s_x80 = ctx.enter_context(nc.semaphore("s_x80"))  # chunk-0 fp8
        s_x = [ctx.enter_context(nc.semaphore(f"s_x{i}")) for i in range(1, NS)]
        s_out = ctx.enter_context(nc.semaphore("s_out"))
        s_pe = ctx.enter_context(nc.semaphore("s_pe"))
        s_act = ctx.enter_context(nc.semaphore("s_act"))
        s_dve = ctx.enter_context(nc.semaphore("s_dve"))

        block = ctx.enter_context(nc.Block())

        # Tick conventions:
        #   s_pe: chunk 0: h groups -> 1..4, g groups -> 5..8;
        #         steady tile t: h -> 2t+1, g -> 2t+2;
        #         epilogue: denominator matmul -> 65, transpose -> 66.
        #   s_act: tile t: tanh -> 2t+1, sigmoid -> 2t+2; exp -> 65.
        #   s_dve: tile t: tw -> 3t+1, z -> 3t+2, reduce -> 3t+3;
        #          epilogue: recip -> 97, alpha -> 98, at copy -> 99.
        def pe_h(t):
            return t + 1 if t < MS else 2 * t + 1

        def pe_g(t):
            return t + 5 if t < MS else 2 * t + 2

        def act_tanh(t):
            return 2 * t + 1

        def act_sig(t):
            return 2 * t + 2

        # ---- DMA programs (SP queue: x stream; ACT queue: weights) ----

        @block.sync
        def _(sync):
            # x chunk 0: bf16 in four 2-ko pieces so the PE can start early,
            # then the fp8 copy (only needed once the g groups run).
            for i in range(4):
                sync.dma_start(
                    xb_sb.ap()[:, 0, 2 * i : 2 * i + 2],
                    xb[:, 0, 2 * i : 2 * i + 2],
                ).then_inc(s_x0[i], 16)
            sync.dma_start(x8_sb.ap()[:, 0], x8[:, 0]).then_inc(s_x80, 16)
            for s in range(1, NS):
                if s >= 2:
                    # x slot s%2 free once PE finished chunk s-2
                    sync.wait_ge(s_pe, 8 * s - 8)
                sync.dma_start(
                    xb_sb.ap()[:, s % 2], xb[:, s]
                ).then_inc(s_x[s - 1], 16)
                sync.dma_start(
                    x8_sb.ap()[:, s % 2], x8[:, s]
                ).then_inc(s_x[s - 1], 16)
            # epilogue constants (needed ~15us after the last x chunk issue)
            sync.dma_start(selb_sb.ap(), selb[:]).then_inc(s_sel, 16)
            sync.dma_start(id_sb.ap(), ident[:]).then_inc(s_id, 16)
            sync.wait_ge(s_dve, 3 * MT + 3)
            sync.dma_start(out[:], at_sb.ap()).then_inc(s_out, 16)
            sync.wait_ge(s_out, 16)

        # ---- PE ----

        @block.tensor
        def _(tensor):
            # Warm-up: fp32 broadcast matmuls keep the PE busy through the
            # DMA-bound startup so the p-state ramp overlaps it.
            c0 = nc.const_aps.aps[(f32, 0.0)]
            c0b = c0.to_broadcast((P, L_DIM))
            for j in range(3):
                nc.tensor.matmul(
                    g_ps.ap()[:1, j, :], c0, c0b, start=True, stop=True
                )

            def h_group(xv, t, chunk0_k=None):
                """Emit the 8 bf16 matmuls of tile t's h group (or, for
                chunk 0 ko-outer staging, just one k)."""
                q = t % MS
                cols = slice(q * P, (q + 1) * P)
                ks = range(KO) if chunk0_k is None else chunk0_k
                for k in ks:
                    mm = nc.tensor.matmul(
                        h_ps.ap()[:, q, :],
                        xv[:, k, cols],
                        v_sb.ap()[:, k, :],
                        start=(k == 0),
                        stop=(k == KO - 1),
                    )
                    if k == KO - 1:
                        mm.then_inc(s_pe, 1)

            def g_group(xv8, t):
                q = t % MS
                cols = slice(q * P, (q + 1) * P)
                for kp in range(KP):
                    mm = nc.tensor.matmul(
                        g_ps.ap()[:, q, :],
                        xv8[:, 2 * kp : 2 * kp + 2, cols],
                        u_sb.ap()[:, 2 * kp : 2 * kp + 2, :],
                        start=(kp == 0),
                        stop=(kp == KP - 1),
                        perf_mode=DR,
                    )
                mm.then_inc(s_pe, 1)

            # ---- chunk 0: ko-outer so compute starts as pieces arrive ----
            xv = xb_sb.ap()[:, 0]
            xv8 = x8_sb.ap()[:, 0]
            for k in range(KO):
                if k % 2 == 0:
                    tensor.wait_ge(s_v[0 if k < 2 else 1], 16)
                    tensor.wait_ge(s_x0[k // 2], 16)
                for q in range(MS):
                    h_group(xv, q, chunk0_k=[k])   # k==7 pass -> ticks 1..4
            tensor.wait_ge(s_u, 16)
            tensor.wait_ge(s_x80, 16)
            for q in range(MS):
                g_group(xv8, q)          # ticks 5..8
            # ---- steady chunks ----
            for t in range(MS, MT):
                s, q = divmod(t, MS)
                xv = xb_sb.ap()[:, s % 2]
                xv8 = x8_sb.ap()[:, s % 2]
                # h bank q free once tanh(t-4) done
                tensor.wait_ge(s_act, act_tanh(t - MS))
                if q == 0:
                    tensor.wait_ge(s_x[s - 1], 32)  # both bf16+fp8 chunks
                h_group(xv, t)           # tick 2t+1
                # g bank q free once sigmoid(t-4) done
                tensor.wait_ge(s_act, act_sig(t - MS))
                g_group(xv8, t)          # tick 2t+2
            # ---- epilogue ----
            tensor.wait_ge(s_sel, 16)
            tensor.wait_ge(s_act, 2 * MT + 1)  # exp/rowsum done; h banks dead
            nc.tensor.matmul(
                rep_ps, selb_sb.ap(), rsum_sb.ap(), start=True, stop=True
            ).then_inc(s_pe, 1)  # -> 65: per-batch sums broadcast to rows
            tensor.wait_ge(s_id, 16)
            tensor.wait_ge(s_dve, 3 * MT + 2)  # alpha ready
            nc.tensor.transpose(at_ps, alpha_sb.ap(), id_sb.ap()).then_inc(
                s_pe, 1
            )  # -> 66

        # ---- ACT ----

        @block.scalar
        def _(scalar):
            # Weight DMAs on the ACT HWDGE queue, in order of first use:
            # v ko0-1 (PE chunk-0 start), v ko2-7, w (first DVE tw),
            # u (g groups).
            scalar.dma_start(v_sb.ap()[:, :2], vb[:, :2]).then_inc(s_v[0], 16)
            scalar.dma_start(v_sb.ap()[:, 2:], vb[:, 2:]).then_inc(s_v[1], 16)
            scalar.dma_start(w_sb.ap(), w_rep[:]).then_inc(s_w, 16)
            scalar.dma_start(u_sb.ap(), uq[:]).then_inc(s_u, 16)
            # Dummy activations: pre-load the tanh/sigmoid tables during
            # the DMA-bound startup.
            c0 = nc.const_aps.aps[(f32, 0.0)]
            for j, fn in enumerate((AF.Tanh, AF.Sigmoid, AF.Exp)):
                nc.scalar.activation(warm_sb.ap()[:, j : j + 1], c0, fn)

            inv = 1.0 / SU
            for t in range(MT):
                q = t % MS
                scalar.wait_ge(s_pe, pe_h(t))
                if t >= MS:
                    scalar.wait_ge(s_dve, 3 * (t - MS) + 1)  # th slot free
                nc.scalar.activation(
                    th_sb.ap()[:, q, :], h_ps.ap()[:, q, :], AF.Tanh
                ).then_inc(s_act, 1)
                scalar.wait_ge(s_pe, pe_g(t))
                if t >= MS:
                    scalar.wait_ge(s_dve, 3 * (t - MS) + 2)  # sg slot free
                nc.scalar.activation(
                    sg_sb.ap()[:, q, :], g_ps.ap()[:, q, :], AF.Sigmoid, scale=inv
                ).then_inc(s_act, 1)
            # Dummy exp BEFORE the final wait: walrus re-emits the exp
            # table load at the sigmoid->exp transition, so trigger it here
            # where it overlaps the DVE tail instead of the critical path.
            nc.scalar.activation(warm_sb.ap()[:, 3:4], c0, AF.Exp)
            # softmax numerators + row sums (no max-subtraction needed:
            # |score| <= sum|w| ~ 28, exp stays well inside fp32 range)
            scalar.wait_ge(s_dve, 3 * MT)  # S complete
            nc.scalar.activation(
                E_sb.ap(), S_sb.ap(), AF.Exp, accum_out=rsum_sb.ap()
            ).then_inc(s_act, 1)  # -> 65

        # ---- DVE ----

        @block.vector
        def _(vector):
            vector.wait_ge(s_w, 16)
            for t in range(MT):
                q = t % MS
                vector.wait_ge(s_act, act_tanh(t))
                if t > 0:
                    vector.wait_ge(s_dve, 3 * t - 1)  # tw WAR vs z(t-1) read
                nc.vector.tensor_tensor(
                    tw_sb.ap(), th_sb.ap()[:, q, :], w_sb.ap(), ALU.mult
                ).then_inc(s_dve, 1)
                vector.wait_ge(s_act, act_sig(t))
                vector.wait_ge(s_dve, 3 * t + 1)  # tw RAW (same-engine order)
                nc.vector.tensor_tensor(
                    z_sb.ap(), tw_sb.ap(), sg_sb.ap()[:, q, :], ALU.mult
                ).then_inc(s_dve, 1)
                vector.wait_ge(s_dve, 3 * t + 2)  # z RAW
                nc.vector.tensor_reduce(
                    S_sb.ap()[:, t : t + 1],
                    z_sb.ap(),
                    axis=mybir.AxisListType.X,
                    op=ALU.add,
                ).then_inc(s_dve, 1)
            # epilogue
            vector.wait_ge(s_pe, 2 * MT + 1)  # rep_ps (denominators) ready
            nc.vector.reciprocal(recip_sb.ap(), rep_ps).then_inc(s_dve, 1)
            vector.wait_ge(s_act, 2 * MT + 1)  # E ready
            vector.wait_ge(s_dve, 3 * MT + 1)  # recip_sb RAW (same engine)
            nc.vector.tensor_scalar_mul(
                alpha_sb.ap(), E_sb.ap(), recip_sb.ap()
            ).then_inc(s_dve, 1)  # -> 98
            vector.wait_ge(s_pe, 2 * MT + 2)  # at_ps ready
            nc.vector.tensor_copy(at_sb.ap(), at_ps).then_inc(s_dve, 1)  # 99

    return nc


def _host_inputs(x, v, u, w):
    """Build the per-core input maps (host-side shard + dtype/layout prep)."""
    import ml_dtypes

    f8 = ml_dtypes.float8_e4m3
    bf = ml_dtypes.bfloat16

    x = np.asarray(x, dtype=np.float32)
    v = np.asarray(v, dtype=np.float32)
    u = np.asarray(u, dtype=np.float32)
    w = np.asarray(w, dtype=np.float32).reshape(L_DIM)

    # [IN_DIM, L] -> [p, ko, L]
    def to_pko(a):
        return np.ascontiguousarray(a.reshape(KO, P, L_DIM).transpose(1, 0, 2))

    vb = to_pko(v.astype(bf))
    uq = to_pko((u * SU).astype(f8))

    w_rep = np.ascontiguousarray(np.broadcast_to(w.astype(bf), (P, L_DIM)))
    selb = (
        np.arange(P)[:, None] % B_LOC == np.arange(P)[None, :] % B_LOC
    ).astype(np.float32)
    ident = np.eye(P, dtype=np.float32)

    common = {
        "vb": vb, "uq": uq, "w_rep": w_rep, "selb": selb, "ident": ident,
    }
    in_maps = []
    for c in range(N_CORES):
        xc = x[:, c * B_LOC : (c + 1) * B_LOC, :].reshape(M, IN_DIM)
        xt = np.ascontiguousarray(xc.T)     # [IN_DIM, M] f32
        # [ko, p, s, m] -> [p, s, ko, m]
        xt4 = xt.reshape(KO, P, NS, MS * P)
        xbc = np.ascontiguousarray(xt4.transpose(1, 2, 0, 3).astype(bf))
        x8c = np.ascontiguousarray(xt4.transpose(1, 2, 0, 3).astype(f8))
        in_maps.append({"xb": xbc, "x8": x8c, **common})
    return in_maps


def kernel(x, v, u, w):
    from concourse.bass_utils import run_bass_kernel_spmd

    if "nc" not in _CACHE:
        _CACHE["nc"] = _build_bass()
    nc = _CACHE["nc"]

    in_maps = _host_inputs(x, v, u, w)
    res = run_bass_kernel_spmd(nc, in_maps, core_ids=list(range(N_CORES)))
    _CACHE["last_result"] = res

    parts = []
    for c in range(N_CORES):
        a = res.results[c]["out"]  # [32, 128], flat index = m = i*16 + b_loc
        parts.append(a.reshape(N_INST, B_LOC))
    full = np.concatenate(parts, axis=1)[:, :, None]
    return np.ascontiguousarray(full.astype(np.float32))


# revision 41
# speedup vs baseline: 1.4441x; 1.0467x over previous
"""Trainium2 Bass kernel for the gated-attention MIL pooling layer.

Computes, for x:[256,128,1024], v,u:[1024,512], w:[512,1]:
    h = tanh(x @ v); g = sigmoid(x @ u)
    scores = (h*g) @ w                      # [256,128,1]
    alpha  = softmax(scores, axis=0)        # over the 256 instances

Sharding: data-parallel over the batch axis (128 -> 16 per core, 8 cores).
Each core handles a [4096,1024]x[1024,512] matmul pair + a local softmax
(softmax is over instances, which live entirely on one core).

Matmul strategy (HW-measured rates: one 512-row matmul instruction costs
~216 ns regardless of dtype; fp8e4m3 DoubleRow contracts K=256 per
instruction vs K=128 for bf16 -> fp8 is 2x bf16):
  - tanh path in bf16 (8 matmuls per 128-row m-tile).  The tanh path
    dominates the softmax error budget (fp8 here measures 2.1e-2 alone,
    over the 2e-2 gate), so it gets the accurate dtype.
  - sigmoid path in plain fp8e4m3 DoubleRow (4 matmuls per m-tile,
    K=256 each via two 128-deep slot pairs).  g-path-only fp8 measures
    7.9e-3 end-to-end - 2.5x under the gate.
  12 instructions per m-tile = 2.6 us -> ~83 us PE per core, vs 110 us
  all-bf16/fp32r.  u is pre-scaled by 16 host-side so its small uniform
  entries stay in e4m3 normal range; the sigmoid applies scale=1/16 on
  the PSUM input (out = func(in*scale)).

Activations write bf16 so the DVE elementwise ops run in 2x mode:
tw = th*w, z = tw*sg, S[:,t] = rowsum(z) (three standard DVE ops; the
fused tensor_tensor_reduce/custom-DVE ops fail this walrus build's
codegen with "ISA wrong length").

Epilogue: exp with accumulated row sums, a selb matmul to broadcast
per-batch softmax denominators, DVE reciprocal + scale, PE transpose,
DMA out.

Raw Bass (explicit per-engine programs + standalone wait_ge semaphores;
the walrus build rejects >1 attached wait per instruction).

DMA: x streams as bf16 (tanh path, 8 MB) + fp8 (sigmoid path, 4 MB) on
the SP queue in 8 chunks (chunk 0 split in four 2-ko bf16 pieces so the
PE can start early); v/w/u go on the ACT HWDGE queue in parallel.
Warm-up matmuls keep the PE p-state ramping through the DMA-bound
startup.
"""

import numpy as np

N_INST, BATCH, IN_DIM, L_DIM = 256, 128, 1024, 512
N_CORES = 8
B_LOC = BATCH // N_CORES            # 16 batch elements per core
M = N_INST * B_LOC                  # 4096 rows per core
P = 128                             # SBUF partitions
KO = IN_DIM // P                    # 8 contraction subtiles
KB = 6                              # h-path subtiles done in bf16 (rest fp8)
KP = KO // 2                        # 4 DoubleRow k-pair groups (g path)
MT = M // P                         # 32 m-tiles per core
MS = 4                              # m-tiles per x DMA chunk
NS = MT // MS                       # 8 DMA chunks
SU = 16.0                           # host-side scale on u,v (e4m3 range)

_CACHE = {}


def _build_bass():
    from contextlib import ExitStack

    import concourse.bass as bass
    import concourse.mybir as mybir

    f32 = mybir.dt.float32
    bf16 = mybir.dt.bfloat16
    f8 = mybir.dt.float8e4
    AF = mybir.ActivationFunctionType
    ALU = mybir.AluOpType
    DR = mybir.MatmulPerfMode.DoubleRow

    nc = bass.Bass(
        trn_type="TRN2",
        target_bir_lowering=False,
        debug=False,
        enable_asserts=False,
    )

    # x in two precisions: bf16 for the tanh path (first KB k-subtiles),
    # fp8 for the sigmoid path and the h tail (all KO subtiles)
    xb = nc.dram_tensor("xb", [P, NS, KB, MS * P], bf16, kind="ExternalInput").ap()
    x8 = nc.dram_tensor("x8", [P, NS, KO, MS * P], f8, kind="ExternalInput").ap()
    vb = nc.dram_tensor("vb", [P, KB, L_DIM], bf16, kind="ExternalInput").ap()
    v8 = nc.dram_tensor("v8", [P, KO - KB, L_DIM], f8, kind="ExternalInput").ap()
    uq = nc.dram_tensor("uq", [P, KO, L_DIM], f8, kind="ExternalInput").ap()
    w_rep = nc.dram_tensor("w_rep", [P, L_DIM], bf16, kind="ExternalInput").ap()
    # selb[r, c] = (r%16 == c%16): one matmul turns the per-row exp sums
    # into per-batch softmax denominators broadcast back to all 128 rows.
    selb = nc.dram_tensor("selb", [P, P], f32, kind="ExternalInput").ap()
    ident = nc.dram_tensor("ident", [P, P], f32, kind="ExternalInput").ap()
    out = nc.dram_tensor("out", [MT, P], f32, kind="ExternalOutput").ap()

    ctx = ExitStack()
    with ctx:
        v_sb = ctx.enter_context(nc.sbuf_tensor("v_sb", [P, KB, L_DIM], bf16))
        v8_sb = ctx.enter_context(
            nc.sbuf_tensor("v8_sb", [P, KO - KB, L_DIM], f8)
        )
        u_sb = ctx.enter_context(nc.sbuf_tensor("u_sb", [P, KO, L_DIM], f8))
        w_sb = ctx.enter_context(nc.sbuf_tensor("w_sb", [P, L_DIM], bf16))
        selb_sb = ctx.enter_context(nc.sbuf_tensor("selb_sb", [P, P], f32))
        id_sb = ctx.enter_context(nc.sbuf_tensor("id_sb", [P, P], f32))
        xb_sb = ctx.enter_context(
            nc.sbuf_tensor("xb_sb", [P, 2, KB, MS * P], bf16)
        )
        x8_sb = ctx.enter_context(
            nc.sbuf_tensor("x8_sb", [P, 2, KO, MS * P], f8)
        )
        th_sb = ctx.enter_context(nc.sbuf_tensor("th_sb", [P, MS, L_DIM], bf16))
        sg_sb = ctx.enter_context(nc.sbuf_tensor("sg_sb", [P, MS, L_DIM], bf16))
        tw_sb = ctx.enter_context(nc.sbuf_tensor("tw_sb", [P, L_DIM], bf16))
        z_sb = ctx.enter_context(nc.sbuf_tensor("z_sb", [P, L_DIM], bf16))
        S_sb = ctx.enter_context(nc.sbuf_tensor("S_sb", [P, MT], f32))
        E_sb = ctx.enter_context(nc.sbuf_tensor("E_sb", [P, MT], f32))
        rsum_sb = ctx.enter_context(nc.sbuf_tensor("rsum_sb", [P, 1], f32))
        recip_sb = ctx.enter_context(nc.sbuf_tensor("recip_sb", [P, 1], f32))
        alpha_sb = ctx.enter_context(nc.sbuf_tensor("alpha_sb", [P, MT], f32))
        at_sb = ctx.enter_context(nc.sbuf_tensor("at_sb", [MT, P], f32))
        warm_sb = ctx.enter_context(nc.sbuf_tensor("warm_sb", [P, 4], f32))

        # All 8 PSUM banks: 4 h accumulation groups + 4 g groups (slot t%4).
        h_ps = ctx.enter_context(nc.psum_tensor("h_ps", [P, MS, L_DIM], f32))
        g_ps = ctx.enter_context(nc.psum_tensor("g_ps", [P, MS, L_DIM], f32))
        # Epilogue PSUM aliases h banks (dead by then; gated on s_act/s_dve).
        rep_ps = h_ps.ap()[:, 1, :1]         # [128, 1] per-batch denominators
        at_ps = h_ps.ap()[:MT, 2, :P]        # [32, 128] transposed alpha

        s_v = [ctx.enter_context(nc.semaphore(f"s_v{i}")) for i in range(3)]
        s_v8 = ctx.enter_context(nc.semaphore("s_v8"))
        s_u = ctx.enter_context(nc.semaphore("s_u"))
        s_w = ctx.enter_context(nc.semaphore("s_w"))
        s_sel = ctx.enter_context(nc.semaphore("s_sel"))
        s_id = ctx.enter_context(nc.semaphore("s_id"))
        s_x0 = [
            ctx.enter_context(nc.semaphore(f"s_x0p{i}")) for i in range(3)
        ]                                                 # chunk-0 bf16 pieces
        s_x80 = ctx.enter_context(nc.semaphore("s_x80"))  # chunk-0 fp8
        s_x = [ctx.enter_context(nc.semaphore(f"s_x{i}")) for i in range(1, NS)]
        s_out = ctx.enter_context(nc.semaphore("s_out"))
        s_pe = ctx.enter_context(nc.semaphore("s_pe"))
        s_act = ctx.enter_context(nc.semaphore("s_act"))
        s_dve = ctx.enter_context(nc.semaphore("s_dve"))

        block = ctx.enter_context(nc.Block())

        # Tick conventions:
        #   s_pe: chunk 0: h groups -> 1..4, g groups -> 5..8;
        #         steady tile t: h -> 2t+1, g -> 2t+2;
        #         epilogue: denominator matmul -> 65, transpose -> 66.
        #   s_act: tile t: tanh -> 2t+1, sigmoid -> 2t+2; exp -> 65.
        #   s_dve: tile t: tw -> 3t+1, z -> 3t+2, reduce -> 3t+3;
        #          epilogue: recip -> 97, alpha -> 98, at copy -> 99.
        def pe_h(t):
            return t + 1 if t < MS else 2 * t + 1

        def pe_g(t):
            return t + 5 if t < MS else 2 * t + 2

        def act_tanh(t):
            return 2 * t + 1

        def act_sig(t):
            return 2 * t + 2

        # ---- DMA programs (SP queue: x stream; ACT queue: weights) ----

        @block.sync
        def _(sync):
            # x chunk 0: bf16 in three 2-ko pieces so the PE can start early,
            # then the fp8 copy (needed by the h tails / g groups).
            for i in range(3):
                sync.dma_start(
                    xb_sb.ap()[:, 0, 2 * i : 2 * i + 2],
                    xb[:, 0, 2 * i : 2 * i + 2],
                ).then_inc(s_x0[i], 16)
            sync.dma_start(x8_sb.ap()[:, 0], x8[:, 0]).then_inc(s_x80, 16)
            for s in range(1, NS):
                if s >= 2:
                    # x slot s%2 free once PE finished chunk s-2
                    sync.wait_ge(s_pe, 8 * s - 8)
                sync.dma_start(
                    xb_sb.ap()[:, s % 2], xb[:, s]
                ).then_inc(s_x[s - 1], 16)
                sync.dma_start(
                    x8_sb.ap()[:, s % 2], x8[:, s]
                ).then_inc(s_x[s - 1], 16)
            # epilogue constants (needed ~15us after the last x chunk issue)
            sync.dma_start(selb_sb.ap(), selb[:]).then_inc(s_sel, 16)
            sync.dma_start(id_sb.ap(), ident[:]).then_inc(s_id, 16)
            sync.wait_ge(s_dve, 3 * MT + 3)  # at copy done
            sync.dma_start(out[:], at_sb.ap()).then_inc(s_out, 16)
            sync.wait_ge(s_out, 16)

        # ---- PE ----

        @block.tensor
        def _(tensor):
            # Warm-up: fp32 broadcast matmuls keep the PE busy through the
            # DMA-bound startup so the p-state ramp overlaps it.
            c0 = nc.const_aps.aps[(f32, 0.0)]
            c0b = c0.to_broadcast((P, L_DIM))
            for j in range(3):
                nc.tensor.matmul(
                    g_ps.ap()[:1, j, :], c0, c0b, start=True, stop=True
                )

            def h_group(xv, t, chunk0_k=None):
                """Emit the KB bf16 matmuls of tile t's h group (or, for
                chunk 0 ko-outer staging, just one k)."""
                q = t % MS
                cols = slice(q * P, (q + 1) * P)
                ks = range(KB) if chunk0_k is None else chunk0_k
                for k in ks:
                    nc.tensor.matmul(
                        h_ps.ap()[:, q, :],
                        xv[:, k, cols],
                        v_sb.ap()[:, k, :],
                        start=(k == 0),
                        stop=False,
                    )

            def h_tail(xv8, t):
                """One fp8 DoubleRow matmul covering the last two h-path
                k-subtiles (KB..KO-1), closing tile t's accumulation group."""
                q = t % MS
                cols = slice(q * P, (q + 1) * P)
                nc.tensor.matmul(
                    h_ps.ap()[:, q, :],
                    xv8[:, KB:KO, cols],
                    v8_sb.ap()[:, :, :],
                    start=False,
                    stop=True,
                    perf_mode=DR,
                ).then_inc(s_pe, 1)

            def g_group(xv8, t):
                q = t % MS
                cols = slice(q * P, (q + 1) * P)
                for kp in range(KP):
                    mm = nc.tensor.matmul(
                        g_ps.ap()[:, q, :],
                        xv8[:, 2 * kp : 2 * kp + 2, cols],
                        u_sb.ap()[:, 2 * kp : 2 * kp + 2, :],
                        start=(kp == 0),
                        stop=(kp == KP - 1),
                        perf_mode=DR,
                    )
                mm.then_inc(s_pe, 1)

            # ---- chunk 0: ko-outer so compute starts as pieces arrive ----
            xv = xb_sb.ap()[:, 0]
            xv8 = x8_sb.ap()[:, 0]
            for k in range(KB):
                if k % 2 == 0:
                    tensor.wait_ge(s_v[k // 2], 16)
                    tensor.wait_ge(s_x0[k // 2], 16)
                for q in range(MS):
                    h_group(xv, q, chunk0_k=[k])
            tensor.wait_ge(s_v8, 16)
            tensor.wait_ge(s_x80, 16)
            for q in range(MS):
                h_tail(xv8, q)           # ticks 1..4
            tensor.wait_ge(s_u, 16)
            for q in range(MS):
                g_group(xv8, q)          # ticks 5..8
            # ---- steady chunks ----
            for t in range(MS, MT):
                s, q = divmod(t, MS)
                xv = xb_sb.ap()[:, s % 2]
                xv8 = x8_sb.ap()[:, s % 2]
                # h bank q free once tanh(t-4) done
                tensor.wait_ge(s_act, act_tanh(t - MS))
                if q == 0:
                    tensor.wait_ge(s_x[s - 1], 32)  # both bf16+fp8 chunks
                h_group(xv, t)
                h_tail(xv8, t)           # tick 2t+1
                # g bank q free once sigmoid(t-4) done
                tensor.wait_ge(s_act, act_sig(t - MS))
                g_group(xv8, t)          # tick 2t+2
            # ---- epilogue ----
            tensor.wait_ge(s_sel, 16)
            tensor.wait_ge(s_act, 2 * MT + 1)  # exp/rowsum done; h banks dead
            nc.tensor.matmul(
                rep_ps, selb_sb.ap(), rsum_sb.ap(), start=True, stop=True
            ).then_inc(s_pe, 1)  # -> 65: per-batch sums broadcast to rows
            tensor.wait_ge(s_id, 16)
            tensor.wait_ge(s_dve, 3 * MT + 2)  # alpha ready (tick 98)
            nc.tensor.transpose(at_ps, alpha_sb.ap(), id_sb.ap()).then_inc(
                s_pe, 1
            )  # -> 66; out DMA reads at_ps directly

        # ---- ACT ----

        @block.scalar
        def _(scalar):
            # Weight DMAs on the ACT HWDGE queue, in order of first use:
            # v in three 2-ko pieces (paced with the PE's chunk-0 ko-outer
            # sweep), v8 (h tails), w (first DVE tw), u (g groups).
            for i in range(3):
                scalar.dma_start(
                    v_sb.ap()[:, 2 * i : 2 * i + 2], vb[:, 2 * i : 2 * i + 2]
                ).then_inc(s_v[i], 16)
            scalar.dma_start(v8_sb.ap(), v8[:]).then_inc(s_v8, 16)
            scalar.dma_start(w_sb.ap(), w_rep[:]).then_inc(s_w, 16)
            scalar.dma_start(u_sb.ap(), uq[:]).then_inc(s_u, 16)
            # Dummy activations: pre-load the tanh/sigmoid tables during
            # the DMA-bound startup.
            c0 = nc.const_aps.aps[(f32, 0.0)]
            for j, fn in enumerate((AF.Tanh, AF.Sigmoid, AF.Exp)):
                nc.scalar.activation(warm_sb.ap()[:, j : j + 1], c0, fn)

            inv = 1.0 / SU
            for t in range(MT):
                q = t % MS
                scalar.wait_ge(s_pe, pe_h(t))
                if t >= MS:
                    scalar.wait_ge(s_dve, 3 * (t - MS) + 1)  # th slot free
                nc.scalar.activation(
                    th_sb.ap()[:, q, :], h_ps.ap()[:, q, :], AF.Tanh, scale=inv
                ).then_inc(s_act, 1)
                scalar.wait_ge(s_pe, pe_g(t))
                if t >= MS:
                    scalar.wait_ge(s_dve, 3 * (t - MS) + 2)  # sg slot free
                nc.scalar.activation(
                    sg_sb.ap()[:, q, :], g_ps.ap()[:, q, :], AF.Sigmoid, scale=inv
                ).then_inc(s_act, 1)
            # Dummy exp BEFORE the final wait: walrus re-emits the exp
            # table load at the sigmoid->exp transition, so trigger it here
            # where it overlaps the DVE tail instead of the critical path.
            nc.scalar.activation(warm_sb.ap()[:, 3:4], c0, AF.Exp)
            # softmax numerators + row sums (no max-subtraction needed:
            # |score| <= sum|w| ~ 28, exp stays well inside fp32 range)
            scalar.wait_ge(s_dve, 3 * MT)  # S complete
            nc.scalar.activation(
                E_sb.ap(), S_sb.ap(), AF.Exp, accum_out=rsum_sb.ap()
            ).then_inc(s_act, 1)  # -> 65

        # ---- DVE ----

        @block.vector
        def _(vector):
            vector.wait_ge(s_w, 16)
            for t in range(MT):
                q = t % MS
                vector.wait_ge(s_act, act_tanh(t))
                if t > 0:
                    vector.wait_ge(s_dve, 3 * t - 1)  # tw WAR vs z(t-1) read
                nc.vector.tensor_tensor(
                    tw_sb.ap(), th_sb.ap()[:, q, :], w_sb.ap(), ALU.mult
                ).then_inc(s_dve, 1)
                vector.wait_ge(s_act, act_sig(t))
                vector.wait_ge(s_dve, 3 * t + 1)  # tw RAW (same-engine order)
                nc.vector.tensor_tensor(
                    z_sb.ap(), tw_sb.ap(), sg_sb.ap()[:, q, :], ALU.mult
                ).then_inc(s_dve, 1)
                vector.wait_ge(s_dve, 3 * t + 2)  # z RAW
                nc.vector.tensor_reduce(
                    S_sb.ap()[:, t : t + 1],
                    z_sb.ap(),
                    axis=mybir.AxisListType.X,
                    op=ALU.add,
                ).then_inc(s_dve, 1)
            # epilogue
            vector.wait_ge(s_pe, 2 * MT + 1)  # rep_ps (denominators) ready
            nc.vector.reciprocal(recip_sb.ap(), rep_ps).then_inc(s_dve, 1)
            vector.wait_ge(s_act, 2 * MT + 1)  # E ready
            vector.wait_ge(s_dve, 3 * MT + 1)  # recip_sb RAW (same engine)
            nc.vector.tensor_scalar_mul(
                alpha_sb.ap(), E_sb.ap(), recip_sb.ap()
            ).then_inc(s_dve, 1)  # -> 98
            vector.wait_ge(s_pe, 2 * MT + 2)  # at_ps ready
            nc.vector.tensor_copy(at_sb.ap(), at_ps).then_inc(s_dve, 1)  # 99

    return nc


def _host_inputs(x, v, u, w):
    """Build the per-core input maps (host-side shard + dtype/layout prep)."""
    import ml_dtypes

    f8 = ml_dtypes.float8_e4m3
    bf = ml_dtypes.bfloat16

    x = np.asarray(x, dtype=np.float32)
    v = np.asarray(v, dtype=np.float32)
    u = np.asarray(u, dtype=np.float32)
    w = np.asarray(w, dtype=np.float32).reshape(L_DIM)

    # [k-subtiles*P, L] -> [p, ko, L]
    def to_pko(a, nk):
        return np.ascontiguousarray(a.reshape(nk, P, L_DIM).transpose(1, 0, 2))

    # v scaled by 16 in BOTH dtypes (exact exponent shift in bf16) so the
    # bf16 and fp8 partial products share one PSUM scale; tanh de-scales.
    vs = v * SU
    vb = to_pko(vs[: KB * P].astype(bf), KB)
    v8 = to_pko(vs[KB * P :].astype(f8), KO - KB)
    uq = to_pko((u * SU).astype(f8), KO)

    w_rep = np.ascontiguousarray(np.broadcast_to(w.astype(bf), (P, L_DIM)))
    selb = (
        np.arange(P)[:, None] % B_LOC == np.arange(P)[None, :] % B_LOC
    ).astype(np.float32)
    ident = np.eye(P, dtype=np.float32)

    common = {
        "vb": vb, "v8": v8, "uq": uq, "w_rep": w_rep,
        "selb": selb, "ident": ident,
    }
    in_maps = []
    for c in range(N_CORES):
        xc = x[:, c * B_LOC : (c + 1) * B_LOC, :].reshape(M, IN_DIM)
        xt = np.ascontiguousarray(xc.T)     # [IN_DIM, M] f32
        # [ko, p, s, m] -> [p, s, ko, m]
        xt4 = xt.reshape(KO, P, NS, MS * P)
        xbc = np.ascontiguousarray(
            xt4[:KB].transpose(1, 2, 0, 3).astype(bf)
        )
        x8c = np.ascontiguousarray(xt4.transpose(1, 2, 0, 3).astype(f8))
        in_maps.append({"xb": xbc, "x8": x8c, **common})
    return in_maps


def kernel(x, v, u, w):
    from concourse.bass_utils import run_bass_kernel_spmd

    if "nc" not in _CACHE:
        _CACHE["nc"] = _build_bass()
    nc = _CACHE["nc"]

    in_maps = _host_inputs(x, v, u, w)
    res = run_bass_kernel_spmd(nc, in_maps, core_ids=list(range(N_CORES)))
    _CACHE["last_result"] = res

    parts = []
    for c in range(N_CORES):
        a = res.results[c]["out"]  # [32, 128], flat index = m = i*16 + b_loc
        parts.append(a.reshape(N_INST, B_LOC))
    full = np.concatenate(parts, axis=1)[:, :, None]
    return np.ascontiguousarray(full.astype(np.float32))


# revision 45
# speedup vs baseline: 1.4539x; 1.0068x over previous
"""Trainium2 Bass kernel for the gated-attention MIL pooling layer.

Computes, for x:[256,128,1024], v,u:[1024,512], w:[512,1]:
    h = tanh(x @ v); g = sigmoid(x @ u)
    scores = (h*g) @ w                      # [256,128,1]
    alpha  = softmax(scores, axis=0)        # over the 256 instances

Sharding: data-parallel over the batch axis (128 -> 16 per core, 8 cores).
Each core handles a [4096,1024]x[1024,512] matmul pair + a local softmax
(softmax is over instances, which live entirely on one core).

Matmul strategy (HW-measured rates: one 512-row matmul instruction costs
~216 ns regardless of dtype; fp8e4m3 DoubleRow contracts K=256 per
instruction vs K=128 for bf16 -> fp8 is 2x bf16):
  - tanh path in bf16 (8 matmuls per 128-row m-tile).  The tanh path
    dominates the softmax error budget (fp8 here measures 2.1e-2 alone,
    over the 2e-2 gate), so it gets the accurate dtype.
  - sigmoid path in plain fp8e4m3 DoubleRow (4 matmuls per m-tile,
    K=256 each via two 128-deep slot pairs).  g-path-only fp8 measures
    7.9e-3 end-to-end - 2.5x under the gate.
  12 instructions per m-tile = 2.6 us -> ~83 us PE per core, vs 110 us
  all-bf16/fp32r.  u is pre-scaled by 16 host-side so its small uniform
  entries stay in e4m3 normal range; the sigmoid applies scale=1/16 on
  the PSUM input (out = func(in*scale)).

Activations write bf16 so the DVE elementwise ops run in 2x mode:
tw = th*w, z = tw*sg, S[:,t] = rowsum(z) (three standard DVE ops; the
fused tensor_tensor_reduce/custom-DVE ops fail this walrus build's
codegen with "ISA wrong length").

Epilogue: exp with accumulated row sums, a selb matmul to broadcast
per-batch softmax denominators, DVE reciprocal + scale, PE transpose,
DMA out.

Raw Bass (explicit per-engine programs + standalone wait_ge semaphores;
the walrus build rejects >1 attached wait per instruction).

DMA: x streams as bf16 (tanh path, 8 MB) + fp8 (sigmoid path, 4 MB) on
the SP queue in 8 chunks (chunk 0 split in four 2-ko bf16 pieces so the
PE can start early); v/w/u go on the ACT HWDGE queue in parallel.
Warm-up matmuls keep the PE p-state ramping through the DMA-bound
startup.
"""

import numpy as np

N_INST, BATCH, IN_DIM, L_DIM = 256, 128, 1024, 512
N_CORES = 8
B_LOC = BATCH // N_CORES            # 16 batch elements per core
M = N_INST * B_LOC                  # 4096 rows per core
P = 128                             # SBUF partitions
KO = IN_DIM // P                    # 8 contraction subtiles
KB = 6                              # h-path subtiles done in bf16 (rest fp8)
KP = KO // 2                        # 4 DoubleRow k-pair groups (g path)
MT = M // P                         # 32 m-tiles per core
MS = 4                              # m-tiles per x DMA chunk
NS = MT // MS                       # 8 DMA chunks
SU = 16.0                           # host-side scale on u,v (e4m3 range)

_CACHE = {}


def _build_bass():
    from contextlib import ExitStack

    import concourse.bass as bass
    import concourse.mybir as mybir

    f32 = mybir.dt.float32
    bf16 = mybir.dt.bfloat16
    f8 = mybir.dt.float8e4
    AF = mybir.ActivationFunctionType
    ALU = mybir.AluOpType
    DR = mybir.MatmulPerfMode.DoubleRow

    nc = bass.Bass(
        trn_type="TRN2",
        target_bir_lowering=False,
        debug=False,
        enable_asserts=False,
    )

    # x in two precisions: bf16 for the tanh path (first KB k-subtiles),
    # fp8 for the sigmoid path and the h tail (all KO subtiles)
    xb = nc.dram_tensor("xb", [P, NS, KB, MS * P], bf16, kind="ExternalInput").ap()
    x8 = nc.dram_tensor("x8", [P, NS, KO, MS * P], f8, kind="ExternalInput").ap()
    vb = nc.dram_tensor("vb", [P, KB, L_DIM], bf16, kind="ExternalInput").ap()
    v8 = nc.dram_tensor("v8", [P, KO - KB, L_DIM], f8, kind="ExternalInput").ap()
    uq = nc.dram_tensor("uq", [P, KO, L_DIM], f8, kind="ExternalInput").ap()
    w_rep = nc.dram_tensor("w_rep", [P, L_DIM], bf16, kind="ExternalInput").ap()
    # selb[r, c] = (r%16 == c%16): one matmul turns the per-row exp sums
    # into per-batch softmax denominators broadcast back to all 128 rows.
    selb = nc.dram_tensor("selb", [P, P], f32, kind="ExternalInput").ap()
    ident = nc.dram_tensor("ident", [P, P], f32, kind="ExternalInput").ap()
    out = nc.dram_tensor("out", [MT, P], f32, kind="ExternalOutput").ap()

    ctx = ExitStack()
    with ctx:
        v_sb = ctx.enter_context(nc.sbuf_tensor("v_sb", [P, KB, L_DIM], bf16))
        v8_sb = ctx.enter_context(
            nc.sbuf_tensor("v8_sb", [P, KO - KB, L_DIM], f8)
        )
        u_sb = ctx.enter_context(nc.sbuf_tensor("u_sb", [P, KO, L_DIM], f8))
        w_sb = ctx.enter_context(nc.sbuf_tensor("w_sb", [P, L_DIM], bf16))
        selb_sb = ctx.enter_context(nc.sbuf_tensor("selb_sb", [P, P], f32))
        id_sb = ctx.enter_context(nc.sbuf_tensor("id_sb", [P, P], f32))
        xb_sb = ctx.enter_context(
            nc.sbuf_tensor("xb_sb", [P, 2, KB, MS * P], bf16)
        )
        x8_sb = ctx.enter_context(
            nc.sbuf_tensor("x8_sb", [P, 2, KO, MS * P], f8)
        )
        th_sb = ctx.enter_context(nc.sbuf_tensor("th_sb", [P, MS, L_DIM], bf16))
        sg_sb = ctx.enter_context(nc.sbuf_tensor("sg_sb", [P, MS, L_DIM], bf16))
        tw_sb = ctx.enter_context(nc.sbuf_tensor("tw_sb", [P, L_DIM], bf16))
        z_sb = ctx.enter_context(nc.sbuf_tensor("z_sb", [P, L_DIM], bf16))
        S_sb = ctx.enter_context(nc.sbuf_tensor("S_sb", [P, MT], f32))
        E_sb = ctx.enter_context(nc.sbuf_tensor("E_sb", [P, MT], f32))
        rsum_sb = ctx.enter_context(nc.sbuf_tensor("rsum_sb", [P, 1], f32))
        recip_sb = ctx.enter_context(nc.sbuf_tensor("recip_sb", [P, 1], f32))
        alpha_sb = ctx.enter_context(nc.sbuf_tensor("alpha_sb", [P, MT], f32))
        at_sb = ctx.enter_context(nc.sbuf_tensor("at_sb", [MT, P], f32))
        warm_sb = ctx.enter_context(nc.sbuf_tensor("warm_sb", [P, 4], f32))

        # All 8 PSUM banks: 4 h accumulation groups + 4 g groups (slot t%4).
        h_ps = ctx.enter_context(nc.psum_tensor("h_ps", [P, MS, L_DIM], f32))
        g_ps = ctx.enter_context(nc.psum_tensor("g_ps", [P, MS, L_DIM], f32))
        # Epilogue PSUM aliases h banks (dead by then; gated on s_act/s_dve).
        rep_ps = h_ps.ap()[:, 1, :1]         # [128, 1] per-batch denominators
        at_ps = h_ps.ap()[:MT, 2, :P]        # [32, 128] transposed alpha

        s_v = [ctx.enter_context(nc.semaphore(f"s_v{i}")) for i in range(3)]
        s_v8 = ctx.enter_context(nc.semaphore("s_v8"))
        s_u = ctx.enter_context(nc.semaphore("s_u"))
        s_w = ctx.enter_context(nc.semaphore("s_w"))
        s_sel = ctx.enter_context(nc.semaphore("s_sel"))
        s_id = ctx.enter_context(nc.semaphore("s_id"))
        s_x0 = [
            ctx.enter_context(nc.semaphore(f"s_x0p{i}")) for i in range(3)
        ]                                                 # chunk-0 bf16 pieces
        s_x80 = ctx.enter_context(nc.semaphore("s_x80"))  # chunk-0 fp8
        s_x = [ctx.enter_context(nc.semaphore(f"s_x{i}")) for i in range(1, NS)]
        s_out = ctx.enter_context(nc.semaphore("s_out"))
        s_pe = ctx.enter_context(nc.semaphore("s_pe"))
        s_act = ctx.enter_context(nc.semaphore("s_act"))
        s_dve = ctx.enter_context(nc.semaphore("s_dve"))

        block = ctx.enter_context(nc.Block())

        # Tick conventions:
        #   s_pe: chunk 0: h groups -> 1..4, g groups -> 5..8;
        #         steady tile t: h -> 2t+1, g -> 2t+2;
        #         epilogue: denominator matmul -> 65, transpose -> 66.
        #   s_act: tile t: tanh -> 2t+1, sigmoid -> 2t+2; exp -> 65.
        #   s_dve: tile t: tw -> 3t+1, z -> 3t+2, reduce -> 3t+3;
        #          epilogue: recip -> 97, alpha -> 98, at copy -> 99.
        def pe_h(t):
            return t + 1 if t < MS else 2 * t + 1

        def pe_g(t):
            return t + 5 if t < MS else 2 * t + 2

        def act_tanh(t):
            return 2 * t + 1

        def act_sig(t):
            return 2 * t + 2

        # ---- DMA programs (SP queue: x stream; ACT queue: weights) ----

        @block.sync
        def _(sync):
            # x chunk 0: bf16 in three 2-ko pieces so the PE can start early,
            # then the fp8 copy (needed by the h tails / g groups).
            for i in range(3):
                sync.dma_start(
                    xb_sb.ap()[:, 0, 2 * i : 2 * i + 2],
                    xb[:, 0, 2 * i : 2 * i + 2],
                ).then_inc(s_x0[i], 16)
            sync.dma_start(x8_sb.ap()[:, 0], x8[:, 0]).then_inc(s_x80, 16)
            for s in range(1, NS):
                if s >= 2:
                    # x slot s%2 free once PE finished chunk s-2
                    sync.wait_ge(s_pe, 8 * s - 8)
                sync.dma_start(
                    xb_sb.ap()[:, s % 2], xb[:, s]
                ).then_inc(s_x[s - 1], 16)
                sync.dma_start(
                    x8_sb.ap()[:, s % 2], x8[:, s]
                ).then_inc(s_x[s - 1], 16)
            # epilogue constants (needed ~15us after the last x chunk issue)
            sync.dma_start(selb_sb.ap(), selb[:]).then_inc(s_sel, 16)
            sync.dma_start(id_sb.ap(), ident[:]).then_inc(s_id, 16)
            sync.wait_ge(s_dve, 3 * MT + 3)  # at copy done
            sync.dma_start(out[:], at_sb.ap()).then_inc(s_out, 16)
            sync.wait_ge(s_out, 16)

        # u on the (otherwise idle) gpsimd SWDGE queue, in parallel with the
        # serial ACT-queue v stream — it was the startup critical path.

        @block.gpsimd
        def _(g):
            g.dma_start(u_sb.ap(), uq[:]).then_inc(s_u, 16)

        # ---- PE ----

        @block.tensor
        def _(tensor):
            # Warm-up: fp32 broadcast matmuls keep the PE busy through the
            # DMA-bound startup so the p-state ramp overlaps it.
            c0 = nc.const_aps.aps[(f32, 0.0)]
            c0b = c0.to_broadcast((P, L_DIM))
            for j in range(3):
                nc.tensor.matmul(
                    g_ps.ap()[:1, j, :], c0, c0b, start=True, stop=True
                )

            def h_group(xv, t, chunk0_k=None):
                """Emit the KB bf16 matmuls of tile t's h group (or, for
                chunk 0 ko-outer staging, just one k)."""
                q = t % MS
                cols = slice(q * P, (q + 1) * P)
                ks = range(KB) if chunk0_k is None else chunk0_k
                for k in ks:
                    nc.tensor.matmul(
                        h_ps.ap()[:, q, :],
                        xv[:, k, cols],
                        v_sb.ap()[:, k, :],
                        start=(k == 0),
                        stop=False,
                    )

            def h_tail(xv8, t):
                """One fp8 DoubleRow matmul covering the last two h-path
                k-subtiles (KB..KO-1), closing tile t's accumulation group."""
                q = t % MS
                cols = slice(q * P, (q + 1) * P)
                nc.tensor.matmul(
                    h_ps.ap()[:, q, :],
                    xv8[:, KB:KO, cols],
                    v8_sb.ap()[:, :, :],
                    start=False,
                    stop=True,
                    perf_mode=DR,
                ).then_inc(s_pe, 1)

            def g_group(xv8, t):
                q = t % MS
                cols = slice(q * P, (q + 1) * P)
                for kp in range(KP):
                    mm = nc.tensor.matmul(
                        g_ps.ap()[:, q, :],
                        xv8[:, 2 * kp : 2 * kp + 2, cols],
                        u_sb.ap()[:, 2 * kp : 2 * kp + 2, :],
                        start=(kp == 0),
                        stop=(kp == KP - 1),
                        perf_mode=DR,
                    )
                mm.then_inc(s_pe, 1)

            # ---- chunk 0: ko-outer so compute starts as pieces arrive ----
            xv = xb_sb.ap()[:, 0]
            xv8 = x8_sb.ap()[:, 0]
            for k in range(KB):
                if k % 2 == 0:
                    tensor.wait_ge(s_v[k // 2], 16)
                    tensor.wait_ge(s_x0[k // 2], 16)
                for q in range(MS):
                    h_group(xv, q, chunk0_k=[k])
            tensor.wait_ge(s_v8, 16)
            tensor.wait_ge(s_x80, 16)
            for q in range(MS):
                h_tail(xv8, q)           # ticks 1..4
            tensor.wait_ge(s_u, 16)
            for q in range(MS):
                g_group(xv8, q)          # ticks 5..8
            # ---- steady chunks ----
            for t in range(MS, MT):
                s, q = divmod(t, MS)
                xv = xb_sb.ap()[:, s % 2]
                xv8 = x8_sb.ap()[:, s % 2]
                # h bank q free once tanh(t-4) done
                tensor.wait_ge(s_act, act_tanh(t - MS))
                if q == 0:
                    tensor.wait_ge(s_x[s - 1], 32)  # both bf16+fp8 chunks
                h_group(xv, t)
                h_tail(xv8, t)           # tick 2t+1
                # g bank q free once sigmoid(t-4) done
                tensor.wait_ge(s_act, act_sig(t - MS))
                g_group(xv8, t)          # tick 2t+2
            # ---- epilogue ----
            tensor.wait_ge(s_sel, 16)
            tensor.wait_ge(s_act, 2 * MT + 1)  # exp/rowsum done; h banks dead
            nc.tensor.matmul(
                rep_ps, selb_sb.ap(), rsum_sb.ap(), start=True, stop=True
            ).then_inc(s_pe, 1)  # -> 65: per-batch sums broadcast to rows
            tensor.wait_ge(s_id, 16)
            tensor.wait_ge(s_dve, 3 * MT + 2)  # alpha ready (tick 98)
            nc.tensor.transpose(at_ps, alpha_sb.ap(), id_sb.ap()).then_inc(
                s_pe, 1
            )  # -> 66; out DMA reads at_ps directly

        # ---- ACT ----

        @block.scalar
        def _(scalar):
            # Weight DMAs on the ACT HWDGE queue, in order of first use:
            # v in three 2-ko pieces (paced with the PE's chunk-0 ko-outer
            # sweep), v8 (h tails), w (first DVE tw), u (g groups).
            for i in range(3):
                scalar.dma_start(
                    v_sb.ap()[:, 2 * i : 2 * i + 2], vb[:, 2 * i : 2 * i + 2]
                ).then_inc(s_v[i], 16)
            scalar.dma_start(v8_sb.ap(), v8[:]).then_inc(s_v8, 16)
            scalar.dma_start(w_sb.ap(), w_rep[:]).then_inc(s_w, 16)
            # Dummy activations: pre-load the tanh/sigmoid tables during
            # the DMA-bound startup.
            c0 = nc.const_aps.aps[(f32, 0.0)]
            for j, fn in enumerate((AF.Tanh, AF.Sigmoid, AF.Exp)):
                nc.scalar.activation(warm_sb.ap()[:, j : j + 1], c0, fn)

            inv = 1.0 / SU
            for t in range(MT):
                q = t % MS
                scalar.wait_ge(s_pe, pe_h(t))
                if t >= MS:
                    scalar.wait_ge(s_dve, 3 * (t - MS) + 1)  # th slot free
                nc.scalar.activation(
                    th_sb.ap()[:, q, :], h_ps.ap()[:, q, :], AF.Tanh, scale=inv
                ).then_inc(s_act, 1)
                scalar.wait_ge(s_pe, pe_g(t))
                if t >= MS:
                    scalar.wait_ge(s_dve, 3 * (t - MS) + 2)  # sg slot free
                nc.scalar.activation(
                    sg_sb.ap()[:, q, :], g_ps.ap()[:, q, :], AF.Sigmoid, scale=inv
                ).then_inc(s_act, 1)
            # Dummy exp BEFORE the final wait: walrus re-emits the exp
            # table load at the sigmoid->exp transition, so trigger it here
            # where it overlaps the DVE tail instead of the critical path.
            nc.scalar.activation(warm_sb.ap()[:, 3:4], c0, AF.Exp)
            # softmax numerators + row sums (no max-subtraction needed:
            # |score| <= sum|w| ~ 28, exp stays well inside fp32 range)
            scalar.wait_ge(s_dve, 3 * MT)  # S complete
            nc.scalar.activation(
                E_sb.ap(), S_sb.ap(), AF.Exp, accum_out=rsum_sb.ap()
            ).then_inc(s_act, 1)  # -> 65

        # ---- DVE ----

        @block.vector
        def _(vector):
            vector.wait_ge(s_w, 16)
            for t in range(MT):
                q = t % MS
                vector.wait_ge(s_act, act_tanh(t))
                if t > 0:
                    vector.wait_ge(s_dve, 3 * t - 1)  # tw WAR vs z(t-1) read
                nc.vector.tensor_tensor(
                    tw_sb.ap(), th_sb.ap()[:, q, :], w_sb.ap(), ALU.mult
                ).then_inc(s_dve, 1)
                vector.wait_ge(s_act, act_sig(t))
                vector.wait_ge(s_dve, 3 * t + 1)  # tw RAW (same-engine order)
                nc.vector.tensor_tensor(
                    z_sb.ap(), tw_sb.ap(), sg_sb.ap()[:, q, :], ALU.mult
                ).then_inc(s_dve, 1)
                vector.wait_ge(s_dve, 3 * t + 2)  # z RAW
                nc.vector.tensor_reduce(
                    S_sb.ap()[:, t : t + 1],
                    z_sb.ap(),
                    axis=mybir.AxisListType.X,
                    op=ALU.add,
                ).then_inc(s_dve, 1)
            # epilogue
            vector.wait_ge(s_pe, 2 * MT + 1)  # rep_ps (denominators) ready
            nc.vector.reciprocal(recip_sb.ap(), rep_ps).then_inc(s_dve, 1)
            vector.wait_ge(s_act, 2 * MT + 1)  # E ready
            vector.wait_ge(s_dve, 3 * MT + 1)  # recip_sb RAW (same engine)
            nc.vector.tensor_scalar_mul(
                alpha_sb.ap(), E_sb.ap(), recip_sb.ap()
            ).then_inc(s_dve, 1)  # -> 98
            vector.wait_ge(s_pe, 2 * MT + 2)  # at_ps ready
            nc.vector.tensor_copy(at_sb.ap(), at_ps).then_inc(s_dve, 1)  # 99

    return nc


def _host_inputs(x, v, u, w):
    """Build the per-core input maps (host-side shard + dtype/layout prep)."""
    import ml_dtypes

    f8 = ml_dtypes.float8_e4m3
    bf = ml_dtypes.bfloat16

    x = np.asarray(x, dtype=np.float32)
    v = np.asarray(v, dtype=np.float32)
    u = np.asarray(u, dtype=np.float32)
    w = np.asarray(w, dtype=np.float32).reshape(L_DIM)

    # [k-subtiles*P, L] -> [p, ko, L]
    def to_pko(a, nk):
        return np.ascontiguousarray(a.reshape(nk, P, L_DIM).transpose(1, 0, 2))

    # v scaled by 16 in BOTH dtypes (exact exponent shift in bf16) so the
    # bf16 and fp8 partial products share one PSUM scale; tanh de-scales.
    vs = v * SU
    vb = to_pko(vs[: KB * P].astype(bf), KB)
    v8 = to_pko(vs[KB * P :].astype(f8), KO - KB)
    uq = to_pko((u * SU).astype(f8), KO)

    w_rep = np.ascontiguousarray(np.broadcast_to(w.astype(bf), (P, L_DIM)))
    selb = (
        np.arange(P)[:, None] % B_LOC == np.arange(P)[None, :] % B_LOC
    ).astype(np.float32)
    ident = np.eye(P, dtype=np.float32)

    common = {
        "vb": vb, "v8": v8, "uq": uq, "w_rep": w_rep,
        "selb": selb, "ident": ident,
    }
    in_maps = []
    for c in range(N_CORES):
        xc = x[:, c * B_LOC : (c + 1) * B_LOC, :].reshape(M, IN_DIM)
        xt = np.ascontiguousarray(xc.T)     # [IN_DIM, M] f32
        # [ko, p, s, m] -> [p, s, ko, m]
        xt4 = xt.reshape(KO, P, NS, MS * P)
        xbc = np.ascontiguousarray(
            xt4[:KB].transpose(1, 2, 0, 3).astype(bf)
        )
        x8c = np.ascontiguousarray(xt4.transpose(1, 2, 0, 3).astype(f8))
        in_maps.append({"xb": xbc, "x8": x8c, **common})
    return in_maps


def kernel(x, v, u, w):
    from concourse.bass_utils import run_bass_kernel_spmd

    if "nc" not in _CACHE:
        _CACHE["nc"] = _build_bass()
    nc = _CACHE["nc"]

    in_maps = _host_inputs(x, v, u, w)
    res = run_bass_kernel_spmd(nc, in_maps, core_ids=list(range(N_CORES)))
    _CACHE["last_result"] = res

    parts = []
    for c in range(N_CORES):
        a = res.results[c]["out"]  # [32, 128], flat index = m = i*16 + b_loc
        parts.append(a.reshape(N_INST, B_LOC))
    full = np.concatenate(parts, axis=1)[:, :, None]
    return np.ascontiguousarray(full.astype(np.float32))


# revision 55
# speedup vs baseline: 1.5040x; 1.0345x over previous
"""Trainium2 Bass kernel for the gated-attention MIL pooling layer.

Computes, for x:[256,128,1024], v,u:[1024,512], w:[512,1]:
    h = tanh(x @ v); g = sigmoid(x @ u)
    scores = (h*g) @ w                      # [256,128,1]
    alpha  = softmax(scores, axis=0)        # over the 256 instances

Sharding: data-parallel over the batch axis (128 -> 16 per core, 8 cores).
Each core handles a [4096,1024]x[1024,512] matmul pair + a local softmax
(softmax is over instances, which live entirely on one core).

Matmul strategy (HW-measured rates: one 512-row matmul instruction costs
~216 ns regardless of dtype; fp8e4m3 DoubleRow contracts K=256 per
instruction vs K=128 for bf16 -> fp8 is 2x bf16):
  - tanh path in bf16 (8 matmuls per 128-row m-tile).  The tanh path
    dominates the softmax error budget (fp8 here measures 2.1e-2 alone,
    over the 2e-2 gate), so it gets the accurate dtype.
  - sigmoid path in plain fp8e4m3 DoubleRow (4 matmuls per m-tile,
    K=256 each via two 128-deep slot pairs).  g-path-only fp8 measures
    7.9e-3 end-to-end - 2.5x under the gate.
  12 instructions per m-tile = 2.6 us -> ~83 us PE per core, vs 110 us
  all-bf16/fp32r.  u is pre-scaled by 16 host-side so its small uniform
  entries stay in e4m3 normal range; the sigmoid applies scale=1/16 on
  the PSUM input (out = func(in*scale)).

Activations write bf16 so the DVE elementwise ops run in 2x mode:
tw = th*w, z = tw*sg, S[:,t] = rowsum(z) (three standard DVE ops; the
fused tensor_tensor_reduce/custom-DVE ops fail this walrus build's
codegen with "ISA wrong length").

Epilogue: exp with accumulated row sums, a selb matmul to broadcast
per-batch softmax denominators, DVE reciprocal + scale, PE transpose,
DMA out.

Raw Bass (explicit per-engine programs + standalone wait_ge semaphores;
the walrus build rejects >1 attached wait per instruction).

DMA: x streams as bf16 (tanh path, 8 MB) + fp8 (sigmoid path, 4 MB) on
the SP queue in 8 chunks (chunk 0 split in four 2-ko bf16 pieces so the
PE can start early); v/w/u go on the ACT HWDGE queue in parallel.
Warm-up matmuls keep the PE p-state ramping through the DMA-bound
startup.
"""

import numpy as np

N_INST, BATCH, IN_DIM, L_DIM = 256, 128, 1024, 512
N_CORES = 8
B_LOC = BATCH // N_CORES            # 16 batch elements per core
M = N_INST * B_LOC                  # 4096 rows per core
P = 128                             # SBUF partitions
KO = IN_DIM // P                    # 8 contraction subtiles
KB = 4                              # h-path subtiles done in bf16 (rest fp8)
KP = KO // 2                        # 4 DoubleRow k-pair groups (g path)
MT = M // P                         # 32 m-tiles per core
MS = 4                              # m-tiles per x DMA chunk
NS = MT // MS                       # 8 DMA chunks
SU = 16.0                           # host-side scale on u,v (e4m3 range)

_CACHE = {}


def _build_bass():
    from contextlib import ExitStack

    import concourse.bass as bass
    import concourse.mybir as mybir

    f32 = mybir.dt.float32
    bf16 = mybir.dt.bfloat16
    f8 = mybir.dt.float8e4
    AF = mybir.ActivationFunctionType
    ALU = mybir.AluOpType
    DR = mybir.MatmulPerfMode.DoubleRow

    nc = bass.Bass(
        trn_type="TRN2",
        target_bir_lowering=False,
        debug=False,
        enable_asserts=False,
    )

    # x in two precisions: bf16 for the tanh path (first KB k-subtiles),
    # fp8 for the sigmoid path and the h tail (all KO subtiles)
    xb = nc.dram_tensor("xb", [P, NS, KB, MS * P], bf16, kind="ExternalInput").ap()
    x8 = nc.dram_tensor("x8", [P, NS, KO, MS * P], f8, kind="ExternalInput").ap()
    vb = nc.dram_tensor("vb", [P, KB, L_DIM], bf16, kind="ExternalInput").ap()
    v8 = nc.dram_tensor("v8", [P, KO - KB, L_DIM], f8, kind="ExternalInput").ap()
    uq = nc.dram_tensor("uq", [P, KO, L_DIM], f8, kind="ExternalInput").ap()
    w_rep = nc.dram_tensor("w_rep", [P, L_DIM], bf16, kind="ExternalInput").ap()
    # selb[r, c] = (r%16 == c%16): one matmul turns the per-row exp sums
    # into per-batch softmax denominators broadcast back to all 128 rows.
    selb = nc.dram_tensor("selb", [P, P], f32, kind="ExternalInput").ap()
    ident = nc.dram_tensor("ident", [P, P], f32, kind="ExternalInput").ap()
    out = nc.dram_tensor("out", [MT, P], f32, kind="ExternalOutput").ap()

    ctx = ExitStack()
    with ctx:
        v_sb = ctx.enter_context(nc.sbuf_tensor("v_sb", [P, KB, L_DIM], bf16))
        v8_sb = ctx.enter_context(
            nc.sbuf_tensor("v8_sb", [P, KO - KB, L_DIM], f8)
        )
        u_sb = ctx.enter_context(nc.sbuf_tensor("u_sb", [P, KO, L_DIM], f8))
        w_sb = ctx.enter_context(nc.sbuf_tensor("w_sb", [P, L_DIM], bf16))
        selb_sb = ctx.enter_context(nc.sbuf_tensor("selb_sb", [P, P], f32))
        id_sb = ctx.enter_context(nc.sbuf_tensor("id_sb", [P, P], f32))
        xb_sb = ctx.enter_context(
            nc.sbuf_tensor("xb_sb", [P, 2, KB, MS * P], bf16)
        )
        x8_sb = ctx.enter_context(
            nc.sbuf_tensor("x8_sb", [P, 2, KO, MS * P], f8)
        )
        th_sb = ctx.enter_context(nc.sbuf_tensor("th_sb", [P, MS, L_DIM], bf16))
        sg_sb = ctx.enter_context(nc.sbuf_tensor("sg_sb", [P, MS, L_DIM], bf16))
        tw_sb = ctx.enter_context(nc.sbuf_tensor("tw_sb", [P, L_DIM], bf16))
        z_sb = ctx.enter_context(nc.sbuf_tensor("z_sb", [P, L_DIM], bf16))
        S_sb = ctx.enter_context(nc.sbuf_tensor("S_sb", [P, MT], f32))
        E_sb = ctx.enter_context(nc.sbuf_tensor("E_sb", [P, MT], f32))
        rsum_sb = ctx.enter_context(nc.sbuf_tensor("rsum_sb", [P, 1], f32))
        recip_sb = ctx.enter_context(nc.sbuf_tensor("recip_sb", [P, 1], f32))
        alpha_sb = ctx.enter_context(nc.sbuf_tensor("alpha_sb", [P, MT], f32))
        at_sb = ctx.enter_context(nc.sbuf_tensor("at_sb", [MT, P], f32))
        warm_sb = ctx.enter_context(nc.sbuf_tensor("warm_sb", [P, 4], f32))

        # All 8 PSUM banks: 4 h accumulation groups + 4 g groups (slot t%4).
        h_ps = ctx.enter_context(nc.psum_tensor("h_ps", [P, MS, L_DIM], f32))
        g_ps = ctx.enter_context(nc.psum_tensor("g_ps", [P, MS, L_DIM], f32))
        # Epilogue PSUM aliases h banks (dead by then; gated on s_act/s_dve).
        rep_ps = h_ps.ap()[:, 1, :1]         # [128, 1] per-batch denominators
        at_ps = h_ps.ap()[:MT, 2, :P]        # [32, 128] transposed alpha

        s_v = [ctx.enter_context(nc.semaphore(f"s_v{i}")) for i in range(2)]
        s_v8 = ctx.enter_context(nc.semaphore("s_v8"))
        s_u = ctx.enter_context(nc.semaphore("s_u"))
        s_w = ctx.enter_context(nc.semaphore("s_w"))
        s_sel = ctx.enter_context(nc.semaphore("s_sel"))
        s_id = ctx.enter_context(nc.semaphore("s_id"))
        s_x0 = [
            ctx.enter_context(nc.semaphore(f"s_x0p{i}")) for i in range(2)
        ]                                                 # chunk-0 bf16 pieces
        s_x80 = ctx.enter_context(nc.semaphore("s_x80"))  # chunk-0 fp8
        s_x = [ctx.enter_context(nc.semaphore(f"s_x{i}")) for i in range(1, NS)]
        s_x8 = [
            ctx.enter_context(nc.semaphore(f"s_x8c{i}")) for i in range(1, NS)
        ]
        s_out = ctx.enter_context(nc.semaphore("s_out"))
        s_pe = ctx.enter_context(nc.semaphore("s_pe"))
        s_act = ctx.enter_context(nc.semaphore("s_act"))
        s_dve = ctx.enter_context(nc.semaphore("s_dve"))

        block = ctx.enter_context(nc.Block())

        # Tick conventions:
        #   s_pe: chunk 0: h groups -> 1..4, g groups -> 5..8;
        #         steady tile t: h -> 2t+1, g -> 2t+2;
        #         epilogue: denominator matmul -> 65, transpose -> 66.
        #   s_act: tile t: tanh -> 2t+1, sigmoid -> 2t+2; exp -> 65.
        #   s_dve: tile t: tw -> 3t+1, z -> 3t+2, reduce -> 3t+3;
        #          epilogue: recip -> 97, alpha -> 98, at copy -> 99.
        def pe_h(t):
            return t + 1 if t < MS else 2 * t + 1

        def pe_g(t):
            return t + 5 if t < MS else 2 * t + 2

        def act_tanh(t):
            return 2 * t + 1

        def act_sig(t):
            return 2 * t + 2

        # ---- DMA programs ----
        # Measured per-queue DMA throughput is only ~115 GB/s (not the
        # modeled 330), so the x stream is split across three queues:
        # SP carries the bf16 x chunks, gpsimd carries u/v8 + all fp8 x
        # chunks (self-gated on s_pe), and the ACT queue carries v/w +
        # the chunk-1 bf16 x.

        @block.sync
        def _(sync):
            # x chunk 0: bf16 in two 2-ko pieces so the PE can start early,
            # then the fp8 copy (needed by the h tails / g groups).
            for i in range(2):
                sync.dma_start(
                    xb_sb.ap()[:, 0, 2 * i : 2 * i + 2],
                    xb[:, 0, 2 * i : 2 * i + 2],
                ).then_inc(s_x0[i], 16)
            sync.dma_start(x8_sb.ap()[:, 0], x8[:, 0]).then_inc(s_x80, 16)
            for s in range(2, NS):
                # x slot s%2 free once PE finished chunk s-2
                sync.wait_ge(s_pe, 8 * s - 8)
                sync.dma_start(
                    xb_sb.ap()[:, s % 2], xb[:, s]
                ).then_inc(s_x[s - 1], 16)
            # epilogue constants (needed ~15us after the last x chunk issue)
            sync.dma_start(selb_sb.ap(), selb[:]).then_inc(s_sel, 16)
            sync.dma_start(id_sb.ap(), ident[:]).then_inc(s_id, 16)
            sync.wait_ge(s_dve, 3 * MT + 3)  # at copy done
            sync.dma_start(out[:], at_sb.ap()).then_inc(s_out, 16)
            sync.wait_ge(s_out, 16)

        @block.gpsimd
        def _(g):
            g.dma_start(u_sb.ap(), uq[:]).then_inc(s_u, 16)
            g.dma_start(v8_sb.ap(), v8[:]).then_inc(s_v8, 16)
            g.dma_start(x8_sb.ap()[:, 1], x8[:, 1]).then_inc(s_x8[0], 16)
            for s in range(2, NS):
                g.wait_ge(s_pe, 8 * s - 8)  # x slot s%2 free
                g.dma_start(
                    x8_sb.ap()[:, s % 2], x8[:, s]
                ).then_inc(s_x8[s - 1], 16)

        # ---- PE ----

        @block.tensor
        def _(tensor):
            # Warm-up: fp32 broadcast matmuls keep the PE busy through the
            # DMA-bound startup so the p-state ramp overlaps it.
            c0 = nc.const_aps.aps[(f32, 0.0)]
            c0b = c0.to_broadcast((P, L_DIM))
            for j in range(3):
                nc.tensor.matmul(
                    g_ps.ap()[:1, j, :], c0, c0b, start=True, stop=True
                )

            def h_group(xv, t, chunk0_k=None):
                """Emit the KB bf16 matmuls of tile t's h group (or, for
                chunk 0 ko-outer staging, just one k)."""
                q = t % MS
                cols = slice(q * P, (q + 1) * P)
                ks = range(KB) if chunk0_k is None else chunk0_k
                for k in ks:
                    nc.tensor.matmul(
                        h_ps.ap()[:, q, :],
                        xv[:, k, cols],
                        v_sb.ap()[:, k, :],
                        start=(k == 0),
                        stop=False,
                    )

            def h_tail(xv8, t):
                """Fp8 DoubleRow matmuls covering h-path k-subtiles KB..KO-1
                (two per instruction), closing tile t's accumulation group."""
                q = t % MS
                cols = slice(q * P, (q + 1) * P)
                npair = (KO - KB) // 2
                for j in range(npair):
                    mm = nc.tensor.matmul(
                        h_ps.ap()[:, q, :],
                        xv8[:, KB + 2 * j : KB + 2 * j + 2, cols],
                        v8_sb.ap()[:, 2 * j : 2 * j + 2, :],
                        start=False,
                        stop=(j == npair - 1),
                        perf_mode=DR,
                    )
                mm.then_inc(s_pe, 1)

            def g_group(xv8, t):
                q = t % MS
                cols = slice(q * P, (q + 1) * P)
                for kp in range(KP):
                    mm = nc.tensor.matmul(
                        g_ps.ap()[:, q, :],
                        xv8[:, 2 * kp : 2 * kp + 2, cols],
                        u_sb.ap()[:, 2 * kp : 2 * kp + 2, :],
                        start=(kp == 0),
                        stop=(kp == KP - 1),
                        perf_mode=DR,
                    )
                mm.then_inc(s_pe, 1)

            # ---- chunk 0: ko-outer so compute starts as pieces arrive ----
            xv = xb_sb.ap()[:, 0]
            xv8 = x8_sb.ap()[:, 0]
            for k in range(KB):
                if k % 2 == 0:
                    tensor.wait_ge(s_v[k // 2], 16)
                    tensor.wait_ge(s_x0[k // 2], 16)
                for q in range(MS):
                    h_group(xv, q, chunk0_k=[k])
            tensor.wait_ge(s_v8, 16)
            tensor.wait_ge(s_x80, 16)
            for q in range(MS):
                h_tail(xv8, q)           # ticks 1..4
            tensor.wait_ge(s_u, 16)
            for q in range(MS):
                g_group(xv8, q)          # ticks 5..8
            # ---- steady chunks ----
            for t in range(MS, MT):
                s, q = divmod(t, MS)
                xv = xb_sb.ap()[:, s % 2]
                xv8 = x8_sb.ap()[:, s % 2]
                # h bank q free once tanh(t-4) done
                tensor.wait_ge(s_act, act_tanh(t - MS))
                if q == 0:
                    tensor.wait_ge(s_x[s - 1], 16)   # bf16 chunk
                    tensor.wait_ge(s_x8[s - 1], 16)  # fp8 chunk
                h_group(xv, t)
                h_tail(xv8, t)           # tick 2t+1
                # g bank q free once sigmoid(t-4) done
                tensor.wait_ge(s_act, act_sig(t - MS))
                g_group(xv8, t)          # tick 2t+2
            # ---- epilogue ----
            tensor.wait_ge(s_sel, 16)
            tensor.wait_ge(s_act, 2 * MT + 1)  # exp/rowsum done; h banks dead
            nc.tensor.matmul(
                rep_ps, selb_sb.ap(), rsum_sb.ap(), start=True, stop=True
            ).then_inc(s_pe, 1)  # -> 65: per-batch sums broadcast to rows
            tensor.wait_ge(s_id, 16)
            tensor.wait_ge(s_dve, 3 * MT + 2)  # alpha ready (tick 98)
            nc.tensor.transpose(at_ps, alpha_sb.ap(), id_sb.ap()).then_inc(
                s_pe, 1
            )  # -> 66; out DMA reads at_ps directly

        # ---- ACT ----

        @block.scalar
        def _(scalar):
            # ACT HWDGE queue: v in two 2-ko pieces (paced with the PE's
            # chunk-0 ko-outer sweep), w (first DVE tw), then the chunk-1
            # bf16 x (SP is still busy with chunk 0 + the fp8 copy).
            for i in range(2):
                scalar.dma_start(
                    v_sb.ap()[:, 2 * i : 2 * i + 2], vb[:, 2 * i : 2 * i + 2]
                ).then_inc(s_v[i], 16)
            scalar.dma_start(w_sb.ap(), w_rep[:]).then_inc(s_w, 16)
            scalar.dma_start(xb_sb.ap()[:, 1], xb[:, 1]).then_inc(s_x[0], 16)
            # Dummy activations: pre-load the tanh/sigmoid tables during
            # the DMA-bound startup.
            c0 = nc.const_aps.aps[(f32, 0.0)]
            for j, fn in enumerate((AF.Tanh, AF.Sigmoid, AF.Exp)):
                nc.scalar.activation(warm_sb.ap()[:, j : j + 1], c0, fn)

            inv = 1.0 / SU
            for t in range(MT):
                q = t % MS
                scalar.wait_ge(s_pe, pe_h(t))
                if t >= MS:
                    scalar.wait_ge(s_dve, 3 * (t - MS) + 1)  # th slot free
                nc.scalar.activation(
                    th_sb.ap()[:, q, :], h_ps.ap()[:, q, :], AF.Tanh, scale=inv
                ).then_inc(s_act, 1)
                scalar.wait_ge(s_pe, pe_g(t))
                if t >= MS:
                    scalar.wait_ge(s_dve, 3 * (t - MS) + 2)  # sg slot free
                nc.scalar.activation(
                    sg_sb.ap()[:, q, :], g_ps.ap()[:, q, :], AF.Sigmoid, scale=inv
                ).then_inc(s_act, 1)
            # Dummy exp BEFORE the final wait: walrus re-emits the exp
            # table load at the sigmoid->exp transition, so trigger it here
            # where it overlaps the DVE tail instead of the critical path.
            nc.scalar.activation(warm_sb.ap()[:, 3:4], c0, AF.Exp)
            # softmax numerators + row sums (no max-subtraction needed:
            # |score| <= sum|w| ~ 28, exp stays well inside fp32 range)
            scalar.wait_ge(s_dve, 3 * MT)  # S complete
            nc.scalar.activation(
                E_sb.ap(), S_sb.ap(), AF.Exp, accum_out=rsum_sb.ap()
            ).then_inc(s_act, 1)  # -> 65

        # ---- DVE ----

        @block.vector
        def _(vector):
            vector.wait_ge(s_w, 16)
            for t in range(MT):
                q = t % MS
                vector.wait_ge(s_act, act_tanh(t))
                if t > 0:
                    vector.wait_ge(s_dve, 3 * t - 1)  # tw WAR vs z(t-1) read
                nc.vector.tensor_tensor(
                    tw_sb.ap(), th_sb.ap()[:, q, :], w_sb.ap(), ALU.mult
                ).then_inc(s_dve, 1)
                vector.wait_ge(s_act, act_sig(t))
                vector.wait_ge(s_dve, 3 * t + 1)  # tw RAW (same-engine order)
                nc.vector.tensor_tensor(
                    z_sb.ap(), tw_sb.ap(), sg_sb.ap()[:, q, :], ALU.mult
                ).then_inc(s_dve, 1)
                vector.wait_ge(s_dve, 3 * t + 2)  # z RAW
                nc.vector.tensor_reduce(
                    S_sb.ap()[:, t : t + 1],
                    z_sb.ap(),
                    axis=mybir.AxisListType.X,
                    op=ALU.add,
                ).then_inc(s_dve, 1)
            # epilogue
            vector.wait_ge(s_pe, 2 * MT + 1)  # rep_ps (denominators) ready
            nc.vector.reciprocal(recip_sb.ap(), rep_ps).then_inc(s_dve, 1)
            vector.wait_ge(s_act, 2 * MT + 1)  # E ready
            vector.wait_ge(s_dve, 3 * MT + 1)  # recip_sb RAW (same engine)
            nc.vector.tensor_scalar_mul(
                alpha_sb.ap(), E_sb.ap(), recip_sb.ap()
            ).then_inc(s_dve, 1)  # -> 98
            vector.wait_ge(s_pe, 2 * MT + 2)  # at_ps ready
            nc.vector.tensor_copy(at_sb.ap(), at_ps).then_inc(s_dve, 1)  # 99

    return nc


def _host_inputs(x, v, u, w):
    """Build the per-core input maps (host-side shard + dtype/layout prep)."""
    import ml_dtypes

    f8 = ml_dtypes.float8_e4m3
    bf = ml_dtypes.bfloat16

    x = np.asarray(x, dtype=np.float32)
    v = np.asarray(v, dtype=np.float32)
    u = np.asarray(u, dtype=np.float32)
    w = np.asarray(w, dtype=np.float32).reshape(L_DIM)

    # [k-subtiles*P, L] -> [p, ko, L]
    def to_pko(a, nk):
        return np.ascontiguousarray(a.reshape(nk, P, L_DIM).transpose(1, 0, 2))

    # v scaled by 16 in BOTH dtypes (exact exponent shift in bf16) so the
    # bf16 and fp8 partial products share one PSUM scale; tanh de-scales.
    vs = v * SU
    vb = to_pko(vs[: KB * P].astype(bf), KB)
    v8 = to_pko(vs[KB * P :].astype(f8), KO - KB)
    uq = to_pko((u * SU).astype(f8), KO)

    w_rep = np.ascontiguousarray(np.broadcast_to(w.astype(bf), (P, L_DIM)))
    selb = (
        np.arange(P)[:, None] % B_LOC == np.arange(P)[None, :] % B_LOC
    ).astype(np.float32)
    ident = np.eye(P, dtype=np.float32)

    common = {
        "vb": vb, "v8": v8, "uq": uq, "w_rep": w_rep,
        "selb": selb, "ident": ident,
    }
    in_maps = []
    for c in range(N_CORES):
        xc = x[:, c * B_LOC : (c + 1) * B_LOC, :].reshape(M, IN_DIM)
        xt = np.ascontiguousarray(xc.T)     # [IN_DIM, M] f32
        # [ko, p, s, m] -> [p, s, ko, m]
        xt4 = xt.reshape(KO, P, NS, MS * P)
        xbc = np.ascontiguousarray(
            xt4[:KB].transpose(1, 2, 0, 3).astype(bf)
        )
        x8c = np.ascontiguousarray(xt4.transpose(1, 2, 0, 3).astype(f8))
        in_maps.append({"xb": xbc, "x8": x8c, **common})
    return in_maps


def kernel(x, v, u, w):
    from concourse.bass_utils import run_bass_kernel_spmd

    if "nc" not in _CACHE:
        _CACHE["nc"] = _build_bass()
    nc = _CACHE["nc"]

    in_maps = _host_inputs(x, v, u, w)
    res = run_bass_kernel_spmd(nc, in_maps, core_ids=list(range(N_CORES)))
    _CACHE["last_result"] = res

    parts = []
    for c in range(N_CORES):
        a = res.results[c]["out"]  # [32, 128], flat index = m = i*16 + b_loc
        parts.append(a.reshape(N_INST, B_LOC))
    full = np.concatenate(parts, axis=1)[:, :, None]
    return np.ascontiguousarray(full.astype(np.float32))
